# revision 47
# baseline (speedup 1.0000x reference)
"""Trainium2 Bass kernel for nn_EncoderLayer (B=2, L=2048, D=1024, 16 heads, FFN 4096).

Strategy: sequence-parallel over the 8 cores (core c owns batch c//4, query rows
(c%4)*512 .. +512).  Each core recomputes the full K projection for its batch,
which avoids all collectives; everything else is local.

v4 (485us -> ~340us HW): fp8 (e4m3) DoubleRow matmuls for the q/k projections,
attn@K, wo, w1 AND w2 — 2x PE throughput (256-wide contraction per 512-cycle
instruction; verified on hw: same 216ns issue rate as a 128-wide bf16 matmul).
Scores stay bf16 (output-column-bound, fp8 gives no gain).  DoubleRow
stationaries must be contiguous [p,256] (walrus ISA check), hence the
co-blocked wq/wk/wo host layouts, the [128, MC/2, NH, 256] kaug layout with
m-chunk pairs adjacent per head, and the pair-interleaved w2 layout.

Precision plan (measured rel err ~1.0e-2 vs 2e-2 gate, matches numpy sim):
weights pre-scaled by powers of 2 (wq,wk,w1 x8; w2 x16) to avoid fp8
subnormals, compensated for free in ACT scale args (exp 1/512, gelu 1/8,
obias 1/16) and the kaug ones-column (=8, cancelling wk's x8 in the softmax
denominator).  Residual/LN paths stay fp32; attention fp8 error is diluted
~100x because the (faithfully reproduced) attn@K-instead-of-V bug makes
attn_out ~1% of x.

Schedule: per-head-pair pipeline (K-proj chunk co -> PE transposes -> scores/
exp/attn@K for heads 2co,2co+1) so ACT exp (~143us total, the attention-phase
floor) fills from ~6us in; PE projection work hides in ACT-bound slack.
Softmax denominators: per-pair reciprocal + DRAM partition-broadcast hidden
under the next pair's compute; the last pair uses a PE selector-matmul
broadcast (psK is free) to keep the DMA round-trip off the critical path into
wo.  LN1's gamma/beta fold into w1/bb1 on the host, so the LN1->FFN seam only
needs sub/mult/fp8-cast per chunk; the residual affine runs inside the FFN
window where ACT is idle.  x-residual and first w1 tiles prefetch during
attention/wo.  Remaining known costs: ~12us LN1 seam, ~16us LN2 tail (serial
stats chains), ~16us startup, DVFS throttling (util limit ~0.72-0.78 under
fp8 load; run-to-run noise +-10us).
"""

import sys
sys.setrecursionlimit(200000)
import numpy as np
import ml_dtypes

B, L, D, NH, HD, FF = 2, 2048, 1024, 16, 64, 4096
LQ = 512  # query rows per core
NCORES = 8
EPS = 1e-5
DC = D // 128  # 8 feature chunks
MC = L // 128  # 16 key chunks
FC = FF // 128  # 32 ffn chunks
BF16NP = ml_dtypes.bfloat16
F8NP = ml_dtypes.float8_e4m3

_cache = {}
LAST_RESULTS = None


def _build_nc():
    import concourse.bass as bass
    import concourse.tile as tile
    from concourse import bacc, mybir
    from contextlib import ExitStack

    f32 = mybir.dt.float32
    bf16 = mybir.dt.bfloat16
    f8 = mybir.dt.float8e4
    AF = mybir.ActivationFunctionType
    OP = mybir.AluOpType
    DR = mybir.MatmulPerfMode.DoubleRow

    nc = bacc.Bacc("TRN2", debug=False, target_bir_lowering=False)

    # ---- DRAM I/O ----
    xb_d = nc.dram_tensor("xb", [4, D, 512], f8, kind="ExternalInput").ap()
    xqb_d = nc.dram_tensor("xqb", [D, LQ], f8, kind="ExternalInput").ap()
    xq_d = nc.dram_tensor("xq", [D, LQ], f32, kind="ExternalInput").ap()
    wq_d = nc.dram_tensor("wq", [DC, 128, D], f8, kind="ExternalInput").ap()
    wk_d = nc.dram_tensor("wk", [DC, 128, D], f8, kind="ExternalInput").ap()
    wo_d = nc.dram_tensor("wo", [DC, 128, D], f8, kind="ExternalInput").ap()
    w1_d = nc.dram_tensor("w1", [FC, 128, D], f8, kind="ExternalInput").ap()
    w2_d = nc.dram_tensor("w2", [FC // 2, 128, 2 * D], f8, kind="ExternalInput").ap()
    ident_d = nc.dram_tensor("ident", [128, 128], bf16, kind="ExternalInput").ap()
    bb1_d = nc.dram_tensor("bb1", [FF], f32, kind="ExternalInput").ap()
    bb2_d = nc.dram_tensor("bb2", [D], f32, kind="ExternalInput").ap()
    g1_d = nc.dram_tensor("g1", [D], f32, kind="ExternalInput").ap()
    b1_d = nc.dram_tensor("b1", [D], f32, kind="ExternalInput").ap()
    g2_d = nc.dram_tensor("g2", [D], f32, kind="ExternalInput").ap()
    b2_d = nc.dram_tensor("b2", [D], f32, kind="ExternalInput").ap()
    out_d = nc.dram_tensor("out", [D, LQ], f32, kind="ExternalOutput").ap()

    xqb_v = xqb_d.rearrange("(c p) l -> p c l", p=128)
    xq_v = xq_d.rearrange("(c p) l -> p c l", p=128)
    bb1_v = bb1_d.rearrange("(c p) -> p c", p=128)
    bb2_v = bb2_d.rearrange("(c p) -> p c", p=128)
    g1_v = g1_d.rearrange("(c p) -> p c", p=128)
    b1_v = b1_d.rearrange("(c p) -> p c", p=128)
    g2_v = g2_d.rearrange("(c p) -> p c", p=128)
    b2_v = b2_d.rearrange("(c p) -> p c", p=128)
    out_v = out_d.rearrange("(c p) l -> p c l", p=128)

    with tile.TileContext(nc, pool_alloc_mode="queue") as tc, ExitStack() as top:
        consts = top.enter_context(tc.tile_pool(name="consts", bufs=1))
        dramsc = top.enter_context(tc.tile_pool(name="dramsc", bufs=2, space="DRAM"))

        sm = top.enter_context(tc.tile_pool(name="smalls", bufs=1))
        sm2 = top.enter_context(tc.tile_pool(name="smalls2", bufs=2))

        with tc.tile_pool(name="mid", bufs=1) as mid:
            hT = mid.tile([128, DC, LQ], f32, tag="hT")
            hb = mid.tile([128, DC, LQ], f8, tag="hb")

            with tc.tile_pool(name="kq", bufs=1) as kq:
                kT = kq.tile([128, DC, L], bf16, tag="kT")
                # kaug[p, mj, h, i*128 + j]: m-chunk pair mj, head h, k-tile i
                # (m = 2*mj+i), col j in [0:64] = head dims, 64 = ones, rest pad
                kaug = kq.tile([128, MC // 2, NH, 256], f8, tag="kaug")
                qT = kq.tile([128, DC, LQ], bf16, tag="qT")
                ctxT = kq.tile([128, DC, LQ], f8, tag="ctxT")

                # ---- Phase 1+2: interleaved projections + attention ----
                with tc.tile_pool(name="p1", bufs=1) as p1, \
                     tc.tile_pool(name="p1w", bufs=1) as p1w, \
                     tc.tile_pool(name="epool", bufs=2) as epool, \
                     tc.tile_pool(name="cpool", bufs=2) as cpool, \
                     tc.tile_pool(name="wop", bufs=1) as wop, \
                     tc.tile_pool(name="psK", bufs=2, space="PSUM") as psK, \
                     tc.tile_pool(name="psT", bufs=1, space="PSUM") as psT, \
                     tc.tile_pool(name="psS", bufs=2, space="PSUM") as psS, \
                     tc.tile_pool(name="psU", bufs=1, space="PSUM") as psU:
    # chunk-0 k-path inputs first so the tensor engine starts early;
                    # the rest of the weight chunks stream behind xb so
                    # head-pair co's inputs land just in time
                    wq_sb = p1w.tile([128, DC, D], f8, tag="wproj")
                    wk_sb = p1w.tile([128, DC, D], f8, tag="wproj_k")
                    xb = p1.tile([128, 4, DC, 512], f8, tag="xb")
                    xqb = p1.tile([128, DC, LQ], f8, tag="xqb")
                    nc.sync.dma_start(xqb, xqb_v)
                    nc.sync.dma_start(wq_sb[:, 0, :], wq_d[0])
                    nc.sync.dma_start(wk_sb[:, 0, :], wk_d[0])
                    for mt in range(4):
                        nc.sync.dma_start(
                            xb[:, mt, :, :],
                            xb_d[mt].rearrange("(c p) m -> p c m", p=128))
                    ident = consts.tile([128, 128], bf16, tag="ident")
                    nc.sync.dma_start(ident, ident_d)
                    for co in range(1, DC):
                        nc.sync.dma_start(wq_sb[:, co, :], wq_d[co])
                        nc.sync.dma_start(wk_sb[:, co, :], wk_d[co])

    # constants (small DMAs, off the critical path)
                    ones_bf = consts.tile([128, 1], bf16, tag="ones")
                    nc.vector.memset(ones_bf, 1.0)
                    # selector rows: sel<s> broadcasts a [1,512] row onto
                    # partitions s*64..s*64+64 via a PE matmul
                    sel0 = consts.tile([1, 128], bf16, tag="sel0")
                    nc.vector.memset(sel0, 0.0)
                    nc.vector.memset(sel0[:, 0:64], 1.0)
                    sel1 = consts.tile([1, 128], bf16, tag="sel1")
                    nc.vector.memset(sel1, 0.0)
                    nc.vector.memset(sel1[:, 64:128], 1.0)
                    ones_row = consts.tile([1, 128], f32, tag="ones_row")
                    nc.vector.memset(ones_row, 1.0)
                    eps_t = consts.tile([1, 1], f32, tag="eps")
                    nc.vector.memset(eps_t, EPS)
                    bb1_sb = consts.tile([128, FC], f32, tag="bb1")
                    nc.sync.dma_start(bb1_sb, bb1_v)
                    bb2_sb = consts.tile([128, DC], f32, tag="bb2")
                    nc.sync.dma_start(bb2_sb, bb2_v)
                    g1_sb = consts.tile([128, DC], f32, tag="g1")
                    nc.sync.dma_start(g1_sb, g1_v)
                    b1_sb = consts.tile([128, DC], f32, tag="b1")
                    nc.sync.dma_start(b1_sb, b1_v)
                    g2_sb = consts.tile([128, DC], f32, tag="g2")
                    nc.sync.dma_start(g2_sb, g2_v)
                    b2_sb = consts.tile([128, DC], f32, tag="b2")
                    nc.sync.dma_start(b2_sb, b2_v)
                    kaug_b = kaug.rearrange("p mj h (two f) -> p mj (h two) f",
                                            two=2)
                    # ones column = 8.0: wk is host-scaled by 8, so kT holds
                    # 8*k; den row becomes 8*sum(e), cancelling the 8 in ctx
                    nc.vector.memset(kaug_b[:, :, :, 64:65], 8.0)
                    nc.vector.memset(kaug_b[:, :, :, 65:128], 0.0)

                    wo_sb = wop.tile([128, DC, D], f8, tag="wo_sb")
                    scd = dramsc.tile([NH, LQ], bf16, tag="rec_sc")

                    for co in range(DC):
                        if co == 2:
                            # prefetch wo once the input stream has drained
                            for cw in range(DC):
                                nc.sync.dma_start(wo_sb[:, cw, :], wo_d[cw])
                        # ---- q chunk co ----
                        psq = psK.tile([128, 512], f32, tag="psk")
                        for cp in range(DC // 2):
                            nc.tensor.matmul(
                                psq,
                                wq_sb[:, co, cp * 256:(cp + 1) * 256]
                                .rearrange("p (two f) -> p two f", two=2),
                                xqb[:, 2 * cp:2 * cp + 2, :],
                                start=(cp == 0), stop=(cp == DC // 2 - 1),
                                perf_mode=DR)
                        nc.vector.tensor_copy(qT[:, co, :], psq)

                        # ---- k chunk co over full L ----
                        for mt in range(4):
                            ps = psK.tile([128, 512], f32, tag="psk")
                            for cp in range(DC // 2):
                                nc.tensor.matmul(
                                    ps,
                                    wk_sb[:, co, cp * 256:(cp + 1) * 256]
                                    .rearrange("p (two f) -> p two f", two=2),
                                    xb[:, mt, 2 * cp:2 * cp + 2, :],
                                    start=(cp == 0), stop=(cp == DC // 2 - 1),
                                    perf_mode=DR)
                            nc.vector.tensor_copy(
                                kT[:, co, mt * 512:(mt + 1) * 512], ps)

                        # ---- transposes -> kaug for heads 2co, 2co+1 ----
                        for g in range(2):
                            pt = psT.tile([128, 1024], bf16, tag="pt")
                            for j in range(8):
                                mi = g * 8 + j
                                nc.tensor.transpose(
                                    pt[:, j * 128:(j + 1) * 128],
                                    kT[:, co, mi * 128:(mi + 1) * 128], ident)
                            ptv = pt.rearrange("p (m he) -> p m he", he=128)
                            for s in range(2):
                                for i in range(2):
                                    # m-chunks g*8+i, g*8+i+2, ... (parity i)
                                    nc.vector.tensor_copy(
                                        kaug[:, g * 4:(g + 1) * 4, 2 * co + s,
                                             i * 128:i * 128 + 64],
                                        ptv[:, i::2, s * 64:(s + 1) * 64])

                        # ---- heads 2co, 2co+1 ----
                        cT = cpool.tile([128, LQ], bf16, tag="cT")
                        den_bc = cpool.tile([128, LQ], bf16, tag="den_bc")
                        rec_pair = []
                        for s in range(2):
                            h = 2 * co + s
                            poff = 64 * s
                            e = epool.tile([128, MC, LQ], f8, tag="E")
                            for mt in range(MC // 2):
                                st = psS.tile([128, 1024], f32, tag="st")
                                for j in range(2):
                                    mi = mt * 2 + j
                                    nc.tensor.matmul(
                                        st[:, j * 512:(j + 1) * 512],
                                        kT[poff:poff + 64, co,
                                           mi * 128:(mi + 1) * 128],
                                        qT[poff:poff + 64, co, :],
                                        start=True, stop=True)
                                # wq,wk host-scaled by 8 => scores are 64x;
                                # fold 1/sqrt(HD)/64 = 1/512 into the exp
                                nc.scalar.activation(
                                    e[:, mt * 2:(mt + 1) * 2, :]
                                    .rearrange("p a b -> p (a b)"),
                                    st, AF.Exp, scale=1.0 / 512.0)
                            u = psU.tile([128, 512], f32, tag="u")
                            for mj in range(MC // 2):
                                nc.tensor.matmul(
                                    u, kaug[:, mj, h, :]
                                    .rearrange("p (two f) -> p two f", two=2),
                                    e[:, 2 * mj:2 * mj + 2, :],
                                    start=(mj == 0), stop=(mj == MC // 2 - 1),
                                    perf_mode=DR)
                            nc.vector.tensor_copy(cT[poff:poff + 64, :],
                                                  u[0:64, :])
                            drow = sm2.tile([1, LQ], f32, tag="drow")
                            nc.vector.tensor_copy(drow, u[64:65, :])
                            rec32 = sm2.tile([1, LQ], f32, tag="rec32")
                            nc.vector.reciprocal_approx_fast(rec32, drow)
                            rec16 = sm2.tile([1, LQ], bf16, tag="rec16")
                            nc.vector.tensor_copy(rec16, rec32)
                            if co < DC - 1:
                                nc.sync.dma_start(scd[h:h + 1, :], rec16)
                                nc.sync.dma_start(
                                    den_bc[poff:poff + 64, :],
                                    scd[h:h + 1, :].partition_broadcast(64))
                            else:
                                # last pair: PE selector broadcast (psK is
                                # free) -- keeps the DMA round-trip off the
                                # critical path into the wo loop
                                rec_pair.append(rec16)
                        if co < DC - 1:
                            nc.vector.tensor_tensor(ctxT[:, co, :], cT, den_bc,
                                                    OP.mult)
                        else:
                            den_ps = psK.tile([128, LQ], f32, tag="psk")
                            nc.tensor.matmul(den_ps, sel0, rec_pair[0],
                                             start=True, stop=False)
                            nc.tensor.matmul(den_ps, sel1, rec_pair[1],
                                             start=False, stop=True)
                            nc.vector.tensor_tensor(ctxT[:, co, :], cT, den_ps,
                                                    OP.mult)

                # ---- attn_out + residual -> r1T, with LN1 prep folded in ----
                with tc.tile_pool(name="r1p", bufs=1) as r1p, \
                     tc.tile_pool(name="psL1", bufs=1, space="PSUM") as psL1, \
                     tc.tile_pool(name="cen1p", bufs=2) as cen1p, \
                     tc.tile_pool(name="psM1", bufs=1, space="PSUM") as psM1:
                    s1_ps = psL1.tile([1, LQ], f32, tag="ln1_sum_r")
                    q1_ps = psL1.tile([1, LQ], f32, tag="ln1_sum_s")
                    r1T = r1p.tile([128, DC, LQ], f32, tag="r1T")
                    xq_all = r1p.tile([128, DC, LQ], f32, tag="xq_all")
                    for cw in range(DC):
                        nc.sync.dma_start(xq_all[:, cw, :], xq_v[:, cw, :])
                    with tc.tile_pool(name="psB", bufs=4, space="PSUM") as psB:
                        for f in range(DC):
                            ps = psB.tile([128, 512], f32, tag="ao")
                            for cp in range(DC // 2):
                                nc.tensor.matmul(
                                    ps,
                                    wo_sb[:, f, cp * 256:(cp + 1) * 256]
                                    .rearrange("p (two f) -> p two f", two=2),
                                    ctxT[:, 2 * cp:2 * cp + 2, :],
                                    start=(cp == 0), stop=(cp == DC // 2 - 1),
                                    perf_mode=DR)
                            nc.vector.tensor_tensor(r1T[:, f, :], ps,
                                                    xq_all[:, f, :], OP.add)
                            rb1 = sm2.tile([128, 512], bf16, tag="rb1")
                            nc.vector.tensor_copy(rb1, r1T[:, f, :])
                            sq1 = sm2.tile([128, 512], bf16, tag="sq1")
                            nc.vector.tensor_tensor(sq1, rb1, rb1, OP.mult)
                            nc.tensor.matmul(s1_ps, ones_bf, rb1,
                                             start=(f == 0), stop=(f == DC - 1))
                            nc.tensor.matmul(q1_ps, ones_bf, sq1,
                                             start=(f == 0), stop=(f == DC - 1))

                    # prefetch the first w1 tiles so the FFN stream starts
                    # as soon as hb chunks appear
                    w1_pre = []
                    for i in range(2):
                        w1t = sm2.tile([128, D], f8, tag="w1pre")
                        nc.sync.dma_start(w1t, w1_d[i])
                        w1_pre.append(w1t)

                    # ---- LN1 stats + normalize (chunkwise) -> hT, hb ----
                    mu = sm.tile([1, LQ], f32, tag="ln_mu")
                    nc.scalar.activation(mu, s1_ps, AF.Copy, scale=1.0 / D)
                    msq = sm.tile([1, LQ], f32, tag="ln_msq")
                    nc.scalar.activation(msq, q1_ps, AF.Copy, scale=1.0 / D)
                    var = sm.tile([1, LQ], f32, tag="ln_var")
                    nc.vector.tensor_tensor(var, mu, mu, OP.mult)
                    nc.vector.tensor_tensor(var, msq, var, OP.subtract)
                    std = sm.tile([1, LQ], f32, tag="ln_std")
                    nc.scalar.activation(std, var, AF.Sqrt, bias=eps_t)
                    mrrow = sm.tile([1, 2 * LQ], f32, tag="ln_mrrow")
                    nc.vector.reciprocal_approx_fast(mrrow[:, LQ:2 * LQ], std)
                    nc.vector.tensor_copy(mrrow[:, 0:LQ], mu)
                    mr_ps = psM1.tile([128, 2 * LQ], f32, tag="ln_mrps")
                    for j in range(2):
                        nc.tensor.matmul(mr_ps[:, j * LQ:(j + 1) * LQ], ones_row,
                                         mrrow[:, j * LQ:(j + 1) * LQ],
                                         start=True, stop=True)
                    mu_bc, rstd_bc = mr_ps[:, 0:LQ], mr_ps[:, LQ:2 * LQ]
                    # hT holds raw cen = (r1-mu)*rstd; g1/b1 are folded into
                    # w1 on the host (w1g, bb1') for the z path, and applied
                    # via an ACT affine inside the FFN window for the residual
                    for c in range(DC):
                        cen = cen1p.tile([128, LQ], f32, tag="ln_cen")
                        nc.vector.tensor_tensor(cen, r1T[:, c, :], mu_bc, OP.subtract)
                        nc.vector.tensor_tensor(hT[:, c, :], cen, rstd_bc, OP.mult)
                        nc.scalar.activation(hb[:, c, :], hT[:, c, :], AF.Copy)
            # ---- Phase 3: FFN ----
            with tc.tile_pool(name="ffn", bufs=1) as ffn, \
                 tc.tile_pool(name="w1stream", bufs=4) as w1stream, \
                 tc.tile_pool(name="w2pool", bufs=1) as w2pool:
                g_sb = ffn.tile([128, FC, LQ], f8, tag="g")
                r2T = ffn.tile([128, DC, LQ], f32, tag="r2T")
                w2_sb = w2pool.tile([128, FC // 2, 2 * D], f8, tag="w2_sb")

                with tc.tile_pool(name="psL2", bufs=1, space="PSUM") as psL2:
                  s2_ps = psL2.tile([1, LQ], f32, tag="ln2_sum_r")
                  q2_ps = psL2.tile([1, LQ], f32, tag="ln2_sum_s")
                  with tc.tile_pool(name="psZO", bufs=1, space="PSUM") as psZO:
                    for half in range(2):
                        o_ps = [psZO.tile([128, 512], f32, tag=f"o{f}", name=f"o_ps{f}")
                                for f in range(4)]
                        for j in range(FC // 2):
                            if half == 0:
                                for i in (2 * j, 2 * j + 1):
                                    if i < 2:
                                        w1t = w1_pre[i]
                                    else:
                                        w1t = w1stream.tile([128, D], f8, tag="w1t")
                                        nc.sync.dma_start(w1t, w1_d[i])
                                    zt = psZO.tile([128, 512], f32, tag=f"zt{i % 2}",
                                                   name=f"zt{i % 2}")
                                    for cp in range(DC // 2):
                                        nc.tensor.matmul(
                                            zt,
                                            w1t[:, cp * 256:(cp + 1) * 256]
                                            .rearrange("p (two f) -> p two f", two=2),
                                            hb[:, 2 * cp:2 * cp + 2, :],
                                            start=(cp == 0), stop=(cp == DC // 2 - 1),
                                            perf_mode=DR)
                                    # w1 host-scaled by 8: z = zt/8 + bb1
                                    nc.scalar.activation(g_sb[:, i, :], zt, AF.Gelu,
                                                         scale=0.125,
                                                         bias=bb1_sb[:, i:i + 1])
                                nc.sync.dma_start(w2_sb[:, j, :], w2_d[j])
                            for f in range(4):
                                fo = half * 4 + f
                                nc.tensor.matmul(
                                    o_ps[f],
                                    w2_sb[:, j, fo * 256:(fo + 1) * 256]
                                    .rearrange("p (two f) -> p two f", two=2),
                                    g_sb[:, 2 * j:2 * j + 2, :],
                                    start=(j == 0), stop=(j == FC // 2 - 1),
                                    perf_mode=DR)
                        for f in range(4):
                            fo = half * 4 + f
                            t = sm2.tile([128, 512], f32, tag="obias")
                            # w2 host-scaled by 16: o = o_ps/16 + bb2
                            nc.scalar.activation(t, o_ps[f], AF.Identity,
                                                 scale=0.0625,
                                                 bias=bb2_sb[:, fo:fo + 1])
                            # h = g1*cen + b1 (deferred LN1 affine)
                            h_aff = sm2.tile([128, 512], f32, tag="h_aff")
                            nc.scalar.activation(h_aff, hT[:, fo, :], AF.Identity,
                                                 scale=g1_sb[:, fo:fo + 1],
                                                 bias=b1_sb[:, fo:fo + 1])
                            nc.vector.tensor_tensor(r2T[:, fo, :], t, h_aff, OP.add)
                            # LN2 prep folded in: bf16 copy + square + partial sums
                            rb2 = sm2.tile([128, 512], bf16, tag="rb2")
                            nc.vector.tensor_copy(rb2, r2T[:, fo, :])
                            sq2 = sm2.tile([128, 512], bf16, tag="sq2")
                            nc.vector.tensor_tensor(sq2, rb2, rb2, OP.mult)
                            nc.tensor.matmul(s2_ps, ones_bf, rb2,
                                             start=(fo == 0), stop=(fo == D // 128 - 1))
                            nc.tensor.matmul(q2_ps, ones_bf, sq2,
                                             start=(fo == 0), stop=(fo == D // 128 - 1))

                  # ---- LN2 stats + normalize -> out (chunked DMA) ----
                  with tc.tile_pool(name="ln2out", bufs=3) as ln2out, \
                       tc.tile_pool(name="psM2", bufs=1, space="PSUM") as psM2:
                      mu = sm.tile([1, LQ], f32, tag="ln_mu")
                      nc.scalar.activation(mu, s2_ps, AF.Copy, scale=1.0 / D)
                      msq = sm.tile([1, LQ], f32, tag="ln_msq")
                      nc.scalar.activation(msq, q2_ps, AF.Copy, scale=1.0 / D)
                      var = sm.tile([1, LQ], f32, tag="ln_var")
                      nc.vector.tensor_tensor(var, mu, mu, OP.mult)
                      nc.vector.tensor_tensor(var, msq, var, OP.subtract)
                      std = sm.tile([1, LQ], f32, tag="ln_std")
                      nc.scalar.activation(std, var, AF.Sqrt, bias=eps_t)
                      mrrow = sm.tile([1, 2 * LQ], f32, tag="ln_mrrow")
                      nc.vector.reciprocal_approx_fast(mrrow[:, LQ:2 * LQ], std)
                      nc.vector.tensor_copy(mrrow[:, 0:LQ], mu)
                      mr_ps = psM2.tile([128, 2 * LQ], f32, tag="ln_mrps")
                      for j in range(2):
                          nc.tensor.matmul(mr_ps[:, j * LQ:(j + 1) * LQ], ones_row,
                                           mrrow[:, j * LQ:(j + 1) * LQ],
                                           start=True, stop=True)
                      mu_bc, rstd_bc = mr_ps[:, 0:LQ], mr_ps[:, LQ:2 * LQ]
                      for c in range(DC):
                          cen = ln2out.tile([128, LQ], f32, tag="ln_cen")
                          nc.vector.tensor_tensor(cen, r2T[:, c, :], mu_bc, OP.subtract)
                          nc.vector.tensor_tensor(cen, cen, rstd_bc, OP.mult)
                          oc = ln2out.tile([128, LQ], f32, tag="ln_oc")
                          nc.scalar.activation(oc, cen, AF.Identity,
                                               scale=g2_sb[:, c:c + 1], bias=b2_sb[:, c:c + 1])
                          nc.sync.dma_start(out_v[:, c, :], oc)

    nc.compile()
    return nc


def _get_nc():
    if "nc" not in _cache:
        _cache["nc"] = _build_nc()
    return _cache["nc"]


def _host_prep(inputs):
    x = np.asarray(inputs["x"], np.float32)
    wq = np.asarray(inputs["wq"], np.float32)
    wk = np.asarray(inputs["wk"], np.float32)
    wo = np.asarray(inputs["wo"], np.float32)
    g1 = np.asarray(inputs["g1"], np.float32)
    b1 = np.asarray(inputs["b1"], np.float32)
    w1 = np.asarray(inputs["w1"], np.float32)
    bb1 = np.asarray(inputs["bb1"], np.float32)
    w2 = np.asarray(inputs["w2"], np.float32)
    bb2 = np.asarray(inputs["bb2"], np.float32)
    g2 = np.asarray(inputs["g2"], np.float32)
    b2 = np.asarray(inputs["b2"], np.float32)

    idx = np.arange(D)
    perm = (idx % HD) * NH + (idx // HD)  # f' = h*64+d  ->  old f = d*16+h

    def bf(a):
        return np.ascontiguousarray(a).astype(BF16NP)

    def f8(a):
        return np.ascontiguousarray(a).astype(F8NP)

    # w1 x8 / w2 x16 / wq,wk x8: power-of-2 pre-scales keep fp8 weights out of
    # the subnormal range; compensated on-device (gelu/obias/exp scale args,
    # kaug ones column = 8).  LN1's gamma/beta fold into w1/bb1 (z = cen@w1g
    # + bb1') so the device feeds raw cen into the FFN.
    w1g = w1 * g1[:, None]
    bb1 = bb1 + b1 @ w1
    w1t = (8.0 * w1g).reshape(DC, 128, FC, 128).transpose(2, 1, 0, 3).reshape(FC, 128, D)
    # w2p[j, p, fc*256 + i*128 + c] = 16*w2[(2j+i)*128+p, fc*128+c]
    w2p = (16.0 * w2).reshape(FC // 2, 2, 128, DC, 128).transpose(0, 2, 3, 1, 4) \
        .reshape(FC // 2, 128, 2 * D)
    def coblock(a):  # [K, F] -> [F-chunk, K-part, K-chunk-major cols]
        return a.reshape(DC, 128, DC, 128).transpose(2, 1, 0, 3).reshape(DC, 128, D)

    shared = {
        "wq": f8(coblock(8.0 * wq[:, perm])),
        "wk": f8(coblock(8.0 * wk[:, perm])),
        "wo": f8(coblock(wo[perm, :])),
        "w1": f8(w1t), "w2": f8(w2p),
        "ident": bf(np.eye(128, dtype=np.float32)),
        "bb1": bb1, "bb2": bb2, "g1": g1, "b1": b1, "g2": g2, "b2": b2,
    }
    in_maps = []
    for c in range(NCORES):
        b, q0 = c // (NCORES // B), (c % (NCORES // B)) * LQ
        xT = np.ascontiguousarray(x[b].T)
        m = dict(shared)
        m["xb"] = f8(np.ascontiguousarray(
            xT.reshape(D, 4, 512).transpose(1, 0, 2)))
        m["xqb"] = f8(xT[:, q0:q0 + LQ])
        m["xq"] = np.ascontiguousarray(xT[:, q0:q0 + LQ])
        in_maps.append(m)
    return in_maps


def kernel(**inputs):
    global LAST_RESULTS
    from concourse.bass_utils import run_bass_kernel_spmd

    nc = _get_nc()
    in_maps = _host_prep(inputs)
    res = run_bass_kernel_spmd(nc, in_maps, core_ids=list(range(NCORES)))
    LAST_RESULTS = res
    out = np.empty((B, L, D), np.float32)
    for c in range(NCORES):
        b, q0 = c // (NCORES // B), (c % (NCORES // B)) * LQ
        out[b, q0:q0 + LQ, :] = res.results[c]["out"].T
    return out


# revision 48
# speedup vs baseline: 1.0246x; 1.0246x over previous
"""Trainium2 Bass kernel for nn_EncoderLayer (B=2, L=2048, D=1024, 16 heads, FFN 4096).

Strategy: sequence-parallel over the 8 cores (core c owns batch c//4, query rows
(c%4)*512 .. +512).  Each core recomputes the full K projection for its batch,
which avoids all collectives; everything else is local.

v4 (485us -> ~340us HW): fp8 (e4m3) DoubleRow matmuls for the q/k projections,
attn@K, wo, w1 AND w2 — 2x PE throughput (256-wide contraction per 512-cycle
instruction; verified on hw: same 216ns issue rate as a 128-wide bf16 matmul).
Scores stay bf16 (output-column-bound, fp8 gives no gain).  DoubleRow
stationaries must be contiguous [p,256] (walrus ISA check), hence the
co-blocked wq/wk/wo host layouts, the [128, MC/2, NH, 256] kaug layout with
m-chunk pairs adjacent per head, and the pair-interleaved w2 layout.

Precision plan (measured rel err ~1.0e-2 vs 2e-2 gate, matches numpy sim):
weights pre-scaled by powers of 2 (wq,wk,w1 x8; w2 x16) to avoid fp8
subnormals, compensated for free in ACT scale args (exp 1/512, gelu 1/8,
obias 1/16) and the kaug ones-column (=8, cancelling wk's x8 in the softmax
denominator).  Residual/LN paths stay fp32; attention fp8 error is diluted
~100x because the (faithfully reproduced) attn@K-instead-of-V bug makes
attn_out ~1% of x.

Schedule: per-head-pair pipeline (K-proj chunk co -> PE transposes -> scores/
exp/attn@K for heads 2co,2co+1) so ACT exp (~143us total, the attention-phase
floor) fills from ~6us in; PE projection work hides in ACT-bound slack.
Softmax denominators: per-pair reciprocal + DRAM partition-broadcast hidden
under the next pair's compute; the last pair uses a PE selector-matmul
broadcast (psK is free) to keep the DMA round-trip off the critical path into
wo.  LN1's gamma/beta fold into w1/bb1 on the host, so the LN1->FFN seam only
needs sub/mult/fp8-cast per chunk; the residual affine runs inside the FFN
window where ACT is idle.  x-residual and first w1 tiles prefetch during
attention/wo.  Remaining known costs: ~12us LN1 seam, ~16us LN2 tail (serial
stats chains), ~16us startup, DVFS throttling (util limit ~0.72-0.78 under
fp8 load; run-to-run noise +-10us).
"""

import sys
sys.setrecursionlimit(200000)
import numpy as np
import ml_dtypes

B, L, D, NH, HD, FF = 2, 2048, 1024, 16, 64, 4096
LQ = 512  # query rows per core
NCORES = 8
EPS = 1e-5
DC = D // 128  # 8 feature chunks
MC = L // 128  # 16 key chunks
FC = FF // 128  # 32 ffn chunks
BF16NP = ml_dtypes.bfloat16
F8NP = ml_dtypes.float8_e4m3

_cache = {}
LAST_RESULTS = None


def _build_nc():
    import concourse.bass as bass
    import concourse.tile as tile
    from concourse import bacc, mybir
    from contextlib import ExitStack

    f32 = mybir.dt.float32
    bf16 = mybir.dt.bfloat16
    f8 = mybir.dt.float8e4
    AF = mybir.ActivationFunctionType
    OP = mybir.AluOpType
    DR = mybir.MatmulPerfMode.DoubleRow

    nc = bacc.Bacc("TRN2", debug=False, target_bir_lowering=False)

    # ---- DRAM I/O ----
    xb_d = nc.dram_tensor("xb", [4, D, 512], f8, kind="ExternalInput").ap()
    xqb_d = nc.dram_tensor("xqb", [D, LQ], f8, kind="ExternalInput").ap()
    xq_d = nc.dram_tensor("xq", [D, LQ], f32, kind="ExternalInput").ap()
    wq_d = nc.dram_tensor("wq", [DC, 128, D], f8, kind="ExternalInput").ap()
    wk_d = nc.dram_tensor("wk", [DC, 128, D], f8, kind="ExternalInput").ap()
    wo_d = nc.dram_tensor("wo", [DC, 128, D], f8, kind="ExternalInput").ap()
    w1_d = nc.dram_tensor("w1", [FC, 128, D], f8, kind="ExternalInput").ap()
    w2_d = nc.dram_tensor("w2", [FC // 2, 128, 2 * D], f8, kind="ExternalInput").ap()
    ident_d = nc.dram_tensor("ident", [128, 128], bf16, kind="ExternalInput").ap()
    bb1_d = nc.dram_tensor("bb1", [FF], f32, kind="ExternalInput").ap()
    bb2_d = nc.dram_tensor("bb2", [D], f32, kind="ExternalInput").ap()
    g1_d = nc.dram_tensor("g1", [D], f32, kind="ExternalInput").ap()
    b1_d = nc.dram_tensor("b1", [D], f32, kind="ExternalInput").ap()
    g2_d = nc.dram_tensor("g2", [D], f32, kind="ExternalInput").ap()
    b2_d = nc.dram_tensor("b2", [D], f32, kind="ExternalInput").ap()
    out_d = nc.dram_tensor("out", [D, LQ], f32, kind="ExternalOutput").ap()

    xqb_v = xqb_d.rearrange("(c p) l -> p c l", p=128)
    xq_v = xq_d.rearrange("(c p) l -> p c l", p=128)
    bb1_v = bb1_d.rearrange("(c p) -> p c", p=128)
    bb2_v = bb2_d.rearrange("(c p) -> p c", p=128)
    g1_v = g1_d.rearrange("(c p) -> p c", p=128)
    b1_v = b1_d.rearrange("(c p) -> p c", p=128)
    g2_v = g2_d.rearrange("(c p) -> p c", p=128)
    b2_v = b2_d.rearrange("(c p) -> p c", p=128)
    out_v = out_d.rearrange("(c p) l -> p c l", p=128)

    with tile.TileContext(nc, pool_alloc_mode="queue") as tc, ExitStack() as top:
        consts = top.enter_context(tc.tile_pool(name="consts", bufs=1))
        dramsc = top.enter_context(tc.tile_pool(name="dramsc", bufs=2, space="DRAM"))

        sm = top.enter_context(tc.tile_pool(name="smalls", bufs=1))
        sm2 = top.enter_context(tc.tile_pool(name="smalls2", bufs=2))

        with tc.tile_pool(name="mid", bufs=1) as mid:
            hT = mid.tile([128, DC, LQ], bf16, tag="hT")
            hb = mid.tile([128, DC, LQ], f8, tag="hb")

            with tc.tile_pool(name="kq", bufs=1) as kq:
                kT = kq.tile([128, DC, L], bf16, tag="kT")
                # kaug[p, mj, h, i*128 + j]: m-chunk pair mj, head h, k-tile i
                # (m = 2*mj+i), col j in [0:64] = head dims, 64 = ones, rest pad
                kaug = kq.tile([128, MC // 2, NH, 256], f8, tag="kaug")
                qT = kq.tile([128, DC, LQ], bf16, tag="qT")
                ctxT = kq.tile([128, DC, LQ], f8, tag="ctxT")

                # ---- Phase 1+2: interleaved projections + attention ----
                with tc.tile_pool(name="p1", bufs=1) as p1, \
                     tc.tile_pool(name="p1w", bufs=1) as p1w, \
                     tc.tile_pool(name="epool", bufs=2) as epool, \
                     tc.tile_pool(name="cpool", bufs=2) as cpool, \
                     tc.tile_pool(name="wop", bufs=1) as wop, \
                     tc.tile_pool(name="psK", bufs=2, space="PSUM") as psK, \
                     tc.tile_pool(name="psT", bufs=1, space="PSUM") as psT, \
                     tc.tile_pool(name="psS", bufs=2, space="PSUM") as psS, \
                     tc.tile_pool(name="psU", bufs=1, space="PSUM") as psU:
    # chunk-0 k-path inputs first so the tensor engine starts early;
                    # the rest of the weight chunks stream behind xb so
                    # head-pair co's inputs land just in time
                    wq_sb = p1w.tile([128, DC, D], f8, tag="wproj")
                    wk_sb = p1w.tile([128, DC, D], f8, tag="wproj_k")
                    xb = p1.tile([128, 4, DC, 512], f8, tag="xb")
                    xqb = p1.tile([128, DC, LQ], f8, tag="xqb")
                    nc.sync.dma_start(xqb, xqb_v)
                    nc.sync.dma_start(wq_sb[:, 0, :], wq_d[0])
                    nc.sync.dma_start(wk_sb[:, 0, :], wk_d[0])
                    for mt in range(4):
                        nc.sync.dma_start(
                            xb[:, mt, :, :],
                            xb_d[mt].rearrange("(c p) m -> p c m", p=128))
                    ident = consts.tile([128, 128], bf16, tag="ident")
                    nc.sync.dma_start(ident, ident_d)
                    for co in range(1, DC):
                        nc.sync.dma_start(wq_sb[:, co, :], wq_d[co])
                        nc.sync.dma_start(wk_sb[:, co, :], wk_d[co])

    # constants (small DMAs, off the critical path)
                    ones_bf = consts.tile([128, 1], bf16, tag="ones")
                    nc.vector.memset(ones_bf, 1.0)
                    # selector rows: sel<s> broadcasts a [1,512] row onto
                    # partitions s*64..s*64+64 via a PE matmul
                    sel0 = consts.tile([1, 128], bf16, tag="sel0")
                    nc.vector.memset(sel0, 0.0)
                    nc.vector.memset(sel0[:, 0:64], 1.0)
                    sel1 = consts.tile([1, 128], bf16, tag="sel1")
                    nc.vector.memset(sel1, 0.0)
                    nc.vector.memset(sel1[:, 64:128], 1.0)
                    ones_row = consts.tile([1, 128], f32, tag="ones_row")
                    nc.vector.memset(ones_row, 1.0)
                    eps_t = consts.tile([1, 1], f32, tag="eps")
                    nc.vector.memset(eps_t, EPS)
                    bb1_sb = consts.tile([128, FC], f32, tag="bb1")
                    nc.sync.dma_start(bb1_sb, bb1_v)
                    bb2_sb = consts.tile([128, DC], f32, tag="bb2")
                    nc.sync.dma_start(bb2_sb, bb2_v)
                    g1_sb = consts.tile([128, DC], f32, tag="g1")
                    nc.sync.dma_start(g1_sb, g1_v)
                    b1_sb = consts.tile([128, DC], f32, tag="b1")
                    nc.sync.dma_start(b1_sb, b1_v)
                    g2_sb = consts.tile([128, DC], f32, tag="g2")
                    nc.sync.dma_start(g2_sb, g2_v)
                    b2_sb = consts.tile([128, DC], f32, tag="b2")
                    nc.sync.dma_start(b2_sb, b2_v)
                    kaug_b = kaug.rearrange("p mj h (two f) -> p mj (h two) f",
                                            two=2)
                    # ones column = 8.0: wk is host-scaled by 8, so kT holds
                    # 8*k; den row becomes 8*sum(e), cancelling the 8 in ctx
                    nc.vector.memset(kaug_b[:, :, :, 64:65], 8.0)
                    nc.vector.memset(kaug_b[:, :, :, 65:128], 0.0)

                    wo_sb = wop.tile([128, DC, D], f8, tag="wo_sb")
                    scd = dramsc.tile([NH, LQ], bf16, tag="rec_sc")

                    for co in range(DC):
                        if co == 2:
                            # prefetch wo once the input stream has drained
                            for cw in range(DC):
                                nc.sync.dma_start(wo_sb[:, cw, :], wo_d[cw])
                        # ---- q chunk co ----
                        psq = psK.tile([128, 512], f32, tag="psk")
                        for cp in range(DC // 2):
                            nc.tensor.matmul(
                                psq,
                                wq_sb[:, co, cp * 256:(cp + 1) * 256]
                                .rearrange("p (two f) -> p two f", two=2),
                                xqb[:, 2 * cp:2 * cp + 2, :],
                                start=(cp == 0), stop=(cp == DC // 2 - 1),
                                perf_mode=DR)
                        nc.vector.tensor_copy(qT[:, co, :], psq)

                        # ---- k chunk co over full L ----
                        for mt in range(4):
                            ps = psK.tile([128, 512], f32, tag="psk")
                            for cp in range(DC // 2):
                                nc.tensor.matmul(
                                    ps,
                                    wk_sb[:, co, cp * 256:(cp + 1) * 256]
                                    .rearrange("p (two f) -> p two f", two=2),
                                    xb[:, mt, 2 * cp:2 * cp + 2, :],
                                    start=(cp == 0), stop=(cp == DC // 2 - 1),
                                    perf_mode=DR)
                            nc.vector.tensor_copy(
                                kT[:, co, mt * 512:(mt + 1) * 512], ps)

                        # ---- transposes -> kaug for heads 2co, 2co+1 ----
                        for g in range(2):
                            pt = psT.tile([128, 1024], bf16, tag="pt")
                            for j in range(8):
                                mi = g * 8 + j
                                nc.tensor.transpose(
                                    pt[:, j * 128:(j + 1) * 128],
                                    kT[:, co, mi * 128:(mi + 1) * 128], ident)
                            ptv = pt.rearrange("p (m he) -> p m he", he=128)
                            for s in range(2):
                                for i in range(2):
                                    # m-chunks g*8+i, g*8+i+2, ... (parity i)
                                    nc.vector.tensor_copy(
                                        kaug[:, g * 4:(g + 1) * 4, 2 * co + s,
                                             i * 128:i * 128 + 64],
                                        ptv[:, i::2, s * 64:(s + 1) * 64])

                        # ---- heads 2co, 2co+1 ----
                        cT = cpool.tile([128, LQ], bf16, tag="cT")
                        den_bc = cpool.tile([128, LQ], bf16, tag="den_bc")
                        rec_pair = []
                        for s in range(2):
                            h = 2 * co + s
                            poff = 64 * s
                            e = epool.tile([128, MC, LQ], f8, tag="E")
                            for mt in range(MC // 2):
                                st = psS.tile([128, 1024], f32, tag="st")
                                for j in range(2):
                                    mi = mt * 2 + j
                                    nc.tensor.matmul(
                                        st[:, j * 512:(j + 1) * 512],
                                        kT[poff:poff + 64, co,
                                           mi * 128:(mi + 1) * 128],
                                        qT[poff:poff + 64, co, :],
                                        start=True, stop=True)
                                # wq,wk host-scaled by 8 => scores are 64x;
                                # fold 1/sqrt(HD)/64 = 1/512 into the exp
                                nc.scalar.activation(
                                    e[:, mt * 2:(mt + 1) * 2, :]
                                    .rearrange("p a b -> p (a b)"),
                                    st, AF.Exp, scale=1.0 / 512.0)
                            u = psU.tile([128, 512], f32, tag="u")
                            for mj in range(MC // 2):
                                nc.tensor.matmul(
                                    u, kaug[:, mj, h, :]
                                    .rearrange("p (two f) -> p two f", two=2),
                                    e[:, 2 * mj:2 * mj + 2, :],
                                    start=(mj == 0), stop=(mj == MC // 2 - 1),
                                    perf_mode=DR)
                            nc.vector.tensor_copy(cT[poff:poff + 64, :],
                                                  u[0:64, :])
                            drow = sm2.tile([1, LQ], f32, tag="drow")
                            nc.vector.tensor_copy(drow, u[64:65, :])
                            rec32 = sm2.tile([1, LQ], f32, tag="rec32")
                            nc.vector.reciprocal_approx_fast(rec32, drow)
                            rec16 = sm2.tile([1, LQ], bf16, tag="rec16")
                            nc.vector.tensor_copy(rec16, rec32)
                            if co < DC - 1:
                                nc.sync.dma_start(scd[h:h + 1, :], rec16)
                                nc.sync.dma_start(
                                    den_bc[poff:poff + 64, :],
                                    scd[h:h + 1, :].partition_broadcast(64))
                            else:
                                # last pair: PE selector broadcast (psK is
                                # free) -- keeps the DMA round-trip off the
                                # critical path into the wo loop
                                rec_pair.append(rec16)
                        if co < DC - 1:
                            nc.vector.tensor_tensor(ctxT[:, co, :], cT, den_bc,
                                                    OP.mult)
                        else:
                            den_ps = psK.tile([128, LQ], f32, tag="psk")
                            nc.tensor.matmul(den_ps, sel0, rec_pair[0],
                                             start=True, stop=False)
                            nc.tensor.matmul(den_ps, sel1, rec_pair[1],
                                             start=False, stop=True)
                            nc.vector.tensor_tensor(ctxT[:, co, :], cT, den_ps,
                                                    OP.mult)

                # ---- attn_out + residual -> r1T, with LN1 prep folded in ----
                with tc.tile_pool(name="r1p", bufs=1) as r1p, \
                     tc.tile_pool(name="psL1", bufs=1, space="PSUM") as psL1, \
                     tc.tile_pool(name="cen1p", bufs=2) as cen1p, \
                     tc.tile_pool(name="psM1", bufs=1, space="PSUM") as psM1:
                    s1_ps = psL1.tile([1, LQ], f32, tag="ln1_sum_r")
                    q1_ps = psL1.tile([1, LQ], f32, tag="ln1_sum_s")
                    r1T = r1p.tile([128, DC, LQ], f32, tag="r1T")
                    r1B = r1p.tile([128, DC, LQ], bf16, tag="r1B")
                    xq_all = r1p.tile([128, DC, LQ], f32, tag="xq_all")
                    for cw in range(DC):
                        nc.sync.dma_start(xq_all[:, cw, :], xq_v[:, cw, :])
                    with tc.tile_pool(name="psB", bufs=4, space="PSUM") as psB:
                        for f in range(DC):
                            ps = psB.tile([128, 512], f32, tag="ao")
                            for cp in range(DC // 2):
                                nc.tensor.matmul(
                                    ps,
                                    wo_sb[:, f, cp * 256:(cp + 1) * 256]
                                    .rearrange("p (two f) -> p two f", two=2),
                                    ctxT[:, 2 * cp:2 * cp + 2, :],
                                    start=(cp == 0), stop=(cp == DC // 2 - 1),
                                    perf_mode=DR)
                            nc.vector.tensor_tensor(r1T[:, f, :], ps,
                                                    xq_all[:, f, :], OP.add)
                            rb1 = r1B[:, f, :]
                            nc.vector.tensor_copy(rb1, r1T[:, f, :])
                            sq1 = sm2.tile([128, 512], bf16, tag="sq1")
                            nc.vector.tensor_tensor(sq1, rb1, rb1, OP.mult)
                            nc.tensor.matmul(s1_ps, ones_bf, rb1,
                                             start=(f == 0), stop=(f == DC - 1))
                            nc.tensor.matmul(q1_ps, ones_bf, sq1,
                                             start=(f == 0), stop=(f == DC - 1))

                    # prefetch the first w1 tiles so the FFN stream starts
                    # as soon as hb chunks appear
                    w1_pre = []
                    for i in range(2):
                        w1t = sm2.tile([128, D], f8, tag="w1pre")
                        nc.sync.dma_start(w1t, w1_d[i])
                        w1_pre.append(w1t)

                    # ---- LN1 stats + normalize (chunkwise) -> hT, hb ----
                    mu = sm.tile([1, LQ], f32, tag="ln_mu")
                    nc.scalar.activation(mu, s1_ps, AF.Copy, scale=1.0 / D)
                    msq = sm.tile([1, LQ], f32, tag="ln_msq")
                    nc.scalar.activation(msq, q1_ps, AF.Copy, scale=1.0 / D)
                    var = sm.tile([1, LQ], f32, tag="ln_var")
                    nc.vector.tensor_tensor(var, mu, mu, OP.mult)
                    nc.vector.tensor_tensor(var, msq, var, OP.subtract)
                    std = sm.tile([1, LQ], f32, tag="ln_std")
                    nc.scalar.activation(std, var, AF.Sqrt, bias=eps_t)
                    mrrow = sm.tile([1, 2 * LQ], f32, tag="ln_mrrow")
                    nc.vector.reciprocal_approx_fast(mrrow[:, LQ:2 * LQ], std)
                    nc.vector.tensor_copy(mrrow[:, 0:LQ], mu)
                    mr_ps = psM1.tile([128, 2 * LQ], f32, tag="ln_mrps")
                    for j in range(2):
                        nc.tensor.matmul(mr_ps[:, j * LQ:(j + 1) * LQ], ones_row,
                                         mrrow[:, j * LQ:(j + 1) * LQ],
                                         start=True, stop=True)
                    mu_bc, rstd_bc = mr_ps[:, 0:LQ], mr_ps[:, LQ:2 * LQ]
                    # hT holds raw cen = (r1-mu)*rstd (bf16); g1/b1 fold into
                    # w1 on the host (w1g, bb1') for the z path, and apply
                    # via an ACT affine inside the FFN window for the residual.
                    # Normalize runs at bf16 DVE width from the r1B copy.
                    mr16 = cen1p.tile([128, 2 * LQ], bf16, tag="mr16")
                    nc.vector.tensor_copy(mr16, mr_ps)
                    for c in range(DC):
                        cen = cen1p.tile([128, LQ], bf16, tag="ln_cen")
                        nc.vector.tensor_tensor(cen, r1B[:, c, :],
                                                mr16[:, 0:LQ], OP.subtract)
                        nc.vector.tensor_tensor(hT[:, c, :], cen,
                                                mr16[:, LQ:2 * LQ], OP.mult)
                        nc.scalar.activation(hb[:, c, :], hT[:, c, :], AF.Copy)
            # ---- Phase 3: FFN ----
            with tc.tile_pool(name="ffn", bufs=1) as ffn, \
                 tc.tile_pool(name="w1stream", bufs=4) as w1stream, \
                 tc.tile_pool(name="w2pool", bufs=1) as w2pool:
                g_sb = ffn.tile([128, FC, LQ], f8, tag="g")
                r2T = ffn.tile([128, DC, LQ], f32, tag="r2T")
                r2B = ffn.tile([128, DC, LQ], bf16, tag="r2B")
                w2_sb = w2pool.tile([128, FC // 2, 2 * D], f8, tag="w2_sb")

                with tc.tile_pool(name="psL2", bufs=1, space="PSUM") as psL2:
                  s2_ps = psL2.tile([1, LQ], f32, tag="ln2_sum_r")
                  q2_ps = psL2.tile([1, LQ], f32, tag="ln2_sum_s")
                  with tc.tile_pool(name="psZO", bufs=1, space="PSUM") as psZO:
                    for half in range(2):
                        o_ps = [psZO.tile([128, 512], f32, tag=f"o{f}", name=f"o_ps{f}")
                                for f in range(4)]
                        for j in range(FC // 2):
                            if half == 0:
                                for i in (2 * j, 2 * j + 1):
                                    if i < 2:
                                        w1t = w1_pre[i]
                                    else:
                                        w1t = w1stream.tile([128, D], f8, tag="w1t")
                                        nc.sync.dma_start(w1t, w1_d[i])
                                    zt = psZO.tile([128, 512], f32, tag=f"zt{i % 2}",
                                                   name=f"zt{i % 2}")
                                    for cp in range(DC // 2):
                                        nc.tensor.matmul(
                                            zt,
                                            w1t[:, cp * 256:(cp + 1) * 256]
                                            .rearrange("p (two f) -> p two f", two=2),
                                            hb[:, 2 * cp:2 * cp + 2, :],
                                            start=(cp == 0), stop=(cp == DC // 2 - 1),
                                            perf_mode=DR)
                                    # w1 host-scaled by 8: z = zt/8 + bb1
                                    nc.scalar.activation(g_sb[:, i, :], zt, AF.Gelu,
                                                         scale=0.125,
                                                         bias=bb1_sb[:, i:i + 1])
                                nc.sync.dma_start(w2_sb[:, j, :], w2_d[j])
                            for f in range(4):
                                fo = half * 4 + f
                                nc.tensor.matmul(
                                    o_ps[f],
                                    w2_sb[:, j, fo * 256:(fo + 1) * 256]
                                    .rearrange("p (two f) -> p two f", two=2),
                                    g_sb[:, 2 * j:2 * j + 2, :],
                                    start=(j == 0), stop=(j == FC // 2 - 1),
                                    perf_mode=DR)
                        for f in range(4):
                            fo = half * 4 + f
                            t = sm2.tile([128, 512], f32, tag="obias")
                            # w2 host-scaled by 16: o = o_ps/16 + bb2
                            nc.scalar.activation(t, o_ps[f], AF.Identity,
                                                 scale=0.0625,
                                                 bias=bb2_sb[:, fo:fo + 1])
                            # h = g1*cen + b1 (deferred LN1 affine)
                            h_aff = sm2.tile([128, 512], f32, tag="h_aff")
                            nc.scalar.activation(h_aff, hT[:, fo, :], AF.Identity,
                                                 scale=g1_sb[:, fo:fo + 1],
                                                 bias=b1_sb[:, fo:fo + 1])
                            nc.vector.tensor_tensor(r2T[:, fo, :], t, h_aff, OP.add)
                            # LN2 prep folded in: bf16 copy + square + partial sums
                            rb2 = r2B[:, fo, :]
                            nc.vector.tensor_copy(rb2, r2T[:, fo, :])
                            sq2 = sm2.tile([128, 512], bf16, tag="sq2")
                            nc.vector.tensor_tensor(sq2, rb2, rb2, OP.mult)
                            nc.tensor.matmul(s2_ps, ones_bf, rb2,
                                             start=(fo == 0), stop=(fo == D // 128 - 1))
                            nc.tensor.matmul(q2_ps, ones_bf, sq2,
                                             start=(fo == 0), stop=(fo == D // 128 - 1))

                  # ---- LN2 stats + normalize -> out (chunked DMA) ----
                  with tc.tile_pool(name="ln2out", bufs=3) as ln2out, \
                       tc.tile_pool(name="psM2", bufs=1, space="PSUM") as psM2:
                      mu = sm.tile([1, LQ], f32, tag="ln_mu")
                      nc.scalar.activation(mu, s2_ps, AF.Copy, scale=1.0 / D)
                      msq = sm.tile([1, LQ], f32, tag="ln_msq")
                      nc.scalar.activation(msq, q2_ps, AF.Copy, scale=1.0 / D)
                      var = sm.tile([1, LQ], f32, tag="ln_var")
                      nc.vector.tensor_tensor(var, mu, mu, OP.mult)
                      nc.vector.tensor_tensor(var, msq, var, OP.subtract)
                      std = sm.tile([1, LQ], f32, tag="ln_std")
                      nc.scalar.activation(std, var, AF.Sqrt, bias=eps_t)
                      mrrow = sm.tile([1, 2 * LQ], f32, tag="ln_mrrow")
                      nc.vector.reciprocal_approx_fast(mrrow[:, LQ:2 * LQ], std)
                      nc.vector.tensor_copy(mrrow[:, 0:LQ], mu)
                      mr_ps = psM2.tile([128, 2 * LQ], f32, tag="ln_mrps")
                      for j in range(2):
                          nc.tensor.matmul(mr_ps[:, j * LQ:(j + 1) * LQ], ones_row,
                                           mrrow[:, j * LQ:(j + 1) * LQ],
                                           start=True, stop=True)
                      mr16b = ln2out.tile([128, 2 * LQ], bf16, tag="mr16b")
                      nc.vector.tensor_copy(mr16b, mr_ps)
                      for c in range(DC):
                          cen = ln2out.tile([128, LQ], bf16, tag="ln_cen")
                          nc.vector.tensor_tensor(cen, r2B[:, c, :],
                                                  mr16b[:, 0:LQ], OP.subtract)
                          nc.vector.tensor_tensor(cen, cen,
                                                  mr16b[:, LQ:2 * LQ], OP.mult)
                          oc = ln2out.tile([128, LQ], f32, tag="ln_oc")
                          nc.scalar.activation(oc, cen, AF.Identity,
                                               scale=g2_sb[:, c:c + 1], bias=b2_sb[:, c:c + 1])
                          nc.sync.dma_start(out_v[:, c, :], oc)

    nc.compile()
    return nc


def _get_nc():
    if "nc" not in _cache:
        _cache["nc"] = _build_nc()
    return _cache["nc"]


def _host_prep(inputs):
    x = np.asarray(inputs["x"], np.float32)
    wq = np.asarray(inputs["wq"], np.float32)
    wk = np.asarray(inputs["wk"], np.float32)
    wo = np.asarray(inputs["wo"], np.float32)
    g1 = np.asarray(inputs["g1"], np.float32)
    b1 = np.asarray(inputs["b1"], np.float32)
    w1 = np.asarray(inputs["w1"], np.float32)
    bb1 = np.asarray(inputs["bb1"], np.float32)
    w2 = np.asarray(inputs["w2"], np.float32)
    bb2 = np.asarray(inputs["bb2"], np.float32)
    g2 = np.asarray(inputs["g2"], np.float32)
    b2 = np.asarray(inputs["b2"], np.float32)

    idx = np.arange(D)
    perm = (idx % HD) * NH + (idx // HD)  # f' = h*64+d  ->  old f = d*16+h

    def bf(a):
        return np.ascontiguousarray(a).astype(BF16NP)

    def f8(a):
        return np.ascontiguousarray(a).astype(F8NP)

    # w1 x8 / w2 x16 / wq,wk x8: power-of-2 pre-scales keep fp8 weights out of
    # the subnormal range; compensated on-device (gelu/obias/exp scale args,
    # kaug ones column = 8).  LN1's gamma/beta fold into w1/bb1 (z = cen@w1g
    # + bb1') so the device feeds raw cen into the FFN.
    w1g = w1 * g1[:, None]
    bb1 = bb1 + b1 @ w1
    w1t = (8.0 * w1g).reshape(DC, 128, FC, 128).transpose(2, 1, 0, 3).reshape(FC, 128, D)
    # w2p[j, p, fc*256 + i*128 + c] = 16*w2[(2j+i)*128+p, fc*128+c]
    w2p = (16.0 * w2).reshape(FC // 2, 2, 128, DC, 128).transpose(0, 2, 3, 1, 4) \
        .reshape(FC // 2, 128, 2 * D)
    def coblock(a):  # [K, F] -> [F-chunk, K-part, K-chunk-major cols]
        return a.reshape(DC, 128, DC, 128).transpose(2, 1, 0, 3).reshape(DC, 128, D)

    shared = {
        "wq": f8(coblock(8.0 * wq[:, perm])),
        "wk": f8(coblock(8.0 * wk[:, perm])),
        "wo": f8(coblock(wo[perm, :])),
        "w1": f8(w1t), "w2": f8(w2p),
        "ident": bf(np.eye(128, dtype=np.float32)),
        "bb1": bb1, "bb2": bb2, "g1": g1, "b1": b1, "g2": g2, "b2": b2,
    }
    in_maps = []
    for c in range(NCORES):
        b, q0 = c // (NCORES // B), (c % (NCORES // B)) * LQ
        xT = np.ascontiguousarray(x[b].T)
        m = dict(shared)
        m["xb"] = f8(np.ascontiguousarray(
            xT.reshape(D, 4, 512).transpose(1, 0, 2)))
        m["xqb"] = f8(xT[:, q0:q0 + LQ])
        m["xq"] = np.ascontiguousarray(xT[:, q0:q0 + LQ])
        in_maps.append(m)
    return in_maps


def kernel(**inputs):
    global LAST_RESULTS
    from concourse.bass_utils import run_bass_kernel_spmd

    nc = _get_nc()
    in_maps = _host_prep(inputs)
    res = run_bass_kernel_spmd(nc, in_maps, core_ids=list(range(NCORES)))
    LAST_RESULTS = res
    out = np.empty((B, L, D), np.float32)
    for c in range(NCORES):
        b, q0 = c // (NCORES // B), (c % (NCORES // B)) * LQ
        out[b, q0:q0 + LQ, :] = res.results[c]["out"].T
    return out


# revision 49
# speedup vs baseline: 1.0324x; 1.0076x over previous
"""Trainium2 Bass kernel for nn_EncoderLayer (B=2, L=2048, D=1024, 16 heads, FFN 4096).

Strategy: sequence-parallel over the 8 cores (core c owns batch c//4, query rows
(c%4)*512 .. +512).  Each core recomputes the full K projection for its batch,
which avoids all collectives; everything else is local.

v4 (485us -> ~340us HW): fp8 (e4m3) DoubleRow matmuls for the q/k projections,
attn@K, wo, w1 AND w2 — 2x PE throughput (256-wide contraction per 512-cycle
instruction; verified on hw: same 216ns issue rate as a 128-wide bf16 matmul).
Scores stay bf16 (output-column-bound, fp8 gives no gain).  DoubleRow
stationaries must be contiguous [p,256] (walrus ISA check), hence the
co-blocked wq/wk/wo host layouts, the [128, MC/2, NH, 256] kaug layout with
m-chunk pairs adjacent per head, and the pair-interleaved w2 layout.

Precision plan (measured rel err ~1.0e-2 vs 2e-2 gate, matches numpy sim):
weights pre-scaled by powers of 2 (wq,wk,w1 x8; w2 x16) to avoid fp8
subnormals, compensated for free in ACT scale args (exp 1/512, gelu 1/8,
obias 1/16) and the kaug ones-column (=8, cancelling wk's x8 in the softmax
denominator).  Residual/LN paths stay fp32; attention fp8 error is diluted
~100x because the (faithfully reproduced) attn@K-instead-of-V bug makes
attn_out ~1% of x.

Schedule: per-head-pair pipeline (K-proj chunk co -> PE transposes -> scores/
exp/attn@K for heads 2co,2co+1) so ACT exp (~143us total, the attention-phase
floor) fills from ~6us in; PE projection work hides in ACT-bound slack.
Softmax denominators: per-pair reciprocal + DRAM partition-broadcast hidden
under the next pair's compute; the last pair uses a PE selector-matmul
broadcast (psK is free) to keep the DMA round-trip off the critical path into
wo.  LN1's gamma/beta fold into w1/bb1 on the host, so the LN1->FFN seam only
needs sub/mult/fp8-cast per chunk; the residual affine runs inside the FFN
window where ACT is idle.  Both LN normalizes run at bf16 DVE width from the
bf16 copies already made for the stats sums (r1B/r2B) — this closed the LN1
seam to ~2us and costs ~0.1% extra rel err.  x-residual and first w1 tiles
prefetch during attention/wo.  Remaining known costs: ~14us LN2 tail (ACT
affine + serial stats chain + output DMA drain), ~16us startup, DVFS
throttling (util limit ~0.72-0.78 under fp8 load; run-to-run noise +-10us).
"""

import sys
sys.setrecursionlimit(200000)
import numpy as np
import ml_dtypes

B, L, D, NH, HD, FF = 2, 2048, 1024, 16, 64, 4096
LQ = 512  # query rows per core
NCORES = 8
EPS = 1e-5
DC = D // 128  # 8 feature chunks
MC = L // 128  # 16 key chunks
FC = FF // 128  # 32 ffn chunks
BF16NP = ml_dtypes.bfloat16
F8NP = ml_dtypes.float8_e4m3

_cache = {}
LAST_RESULTS = None


def _build_nc():
    import concourse.bass as bass
    import concourse.tile as tile
    from concourse import bacc, mybir
    from contextlib import ExitStack

    f32 = mybir.dt.float32
    bf16 = mybir.dt.bfloat16
    f8 = mybir.dt.float8e4
    AF = mybir.ActivationFunctionType
    OP = mybir.AluOpType
    DR = mybir.MatmulPerfMode.DoubleRow

    nc = bacc.Bacc("TRN2", debug=False, target_bir_lowering=False)

    # ---- DRAM I/O ----
    xb_d = nc.dram_tensor("xb", [4, D, 512], f8, kind="ExternalInput").ap()
    xqb_d = nc.dram_tensor("xqb", [D, LQ], f8, kind="ExternalInput").ap()
    xq_d = nc.dram_tensor("xq", [D, LQ], f32, kind="ExternalInput").ap()
    wq_d = nc.dram_tensor("wq", [DC, 128, D], f8, kind="ExternalInput").ap()
    wk_d = nc.dram_tensor("wk", [DC, 128, D], f8, kind="ExternalInput").ap()
    wo_d = nc.dram_tensor("wo", [DC, 128, D], f8, kind="ExternalInput").ap()
    w1_d = nc.dram_tensor("w1", [FC, 128, D], f8, kind="ExternalInput").ap()
    w2_d = nc.dram_tensor("w2", [FC // 2, 128, 2 * D], f8, kind="ExternalInput").ap()
    ident_d = nc.dram_tensor("ident", [128, 128], bf16, kind="ExternalInput").ap()
    bb1_d = nc.dram_tensor("bb1", [FF], f32, kind="ExternalInput").ap()
    bb2_d = nc.dram_tensor("bb2", [D], f32, kind="ExternalInput").ap()
    g1_d = nc.dram_tensor("g1", [D], f32, kind="ExternalInput").ap()
    b1_d = nc.dram_tensor("b1", [D], f32, kind="ExternalInput").ap()
    g2_d = nc.dram_tensor("g2", [D], f32, kind="ExternalInput").ap()
    b2_d = nc.dram_tensor("b2", [D], f32, kind="ExternalInput").ap()
    out_d = nc.dram_tensor("out", [D, LQ], f32, kind="ExternalOutput").ap()

    xqb_v = xqb_d.rearrange("(c p) l -> p c l", p=128)
    xq_v = xq_d.rearrange("(c p) l -> p c l", p=128)
    bb1_v = bb1_d.rearrange("(c p) -> p c", p=128)
    bb2_v = bb2_d.rearrange("(c p) -> p c", p=128)
    g1_v = g1_d.rearrange("(c p) -> p c", p=128)
    b1_v = b1_d.rearrange("(c p) -> p c", p=128)
    g2_v = g2_d.rearrange("(c p) -> p c", p=128)
    b2_v = b2_d.rearrange("(c p) -> p c", p=128)
    out_v = out_d.rearrange("(c p) l -> p c l", p=128)

    with tile.TileContext(nc, pool_alloc_mode="queue") as tc, ExitStack() as top:
        consts = top.enter_context(tc.tile_pool(name="consts", bufs=1))
        dramsc = top.enter_context(tc.tile_pool(name="dramsc", bufs=2, space="DRAM"))

        sm = top.enter_context(tc.tile_pool(name="smalls", bufs=1))
        sm2 = top.enter_context(tc.tile_pool(name="smalls2", bufs=2))

        with tc.tile_pool(name="mid", bufs=1) as mid:
            hT = mid.tile([128, DC, LQ], bf16, tag="hT")
            hb = mid.tile([128, DC, LQ], f8, tag="hb")

            with tc.tile_pool(name="kq", bufs=1) as kq:
                kT = kq.tile([128, DC, L], bf16, tag="kT")
                # kaug[p, mj, h, i*128 + j]: m-chunk pair mj, head h, k-tile i
                # (m = 2*mj+i), col j in [0:64] = head dims, 64 = ones, rest pad
                kaug = kq.tile([128, MC // 2, NH, 256], f8, tag="kaug")
                qT = kq.tile([128, DC, LQ], bf16, tag="qT")
                ctxT = kq.tile([128, DC, LQ], f8, tag="ctxT")

                # ---- Phase 1+2: interleaved projections + attention ----
                with tc.tile_pool(name="p1", bufs=1) as p1, \
                     tc.tile_pool(name="p1w", bufs=1) as p1w, \
                     tc.tile_pool(name="epool", bufs=2) as epool, \
                     tc.tile_pool(name="cpool", bufs=2) as cpool, \
                     tc.tile_pool(name="wop", bufs=1) as wop, \
                     tc.tile_pool(name="psK", bufs=2, space="PSUM") as psK, \
                     tc.tile_pool(name="psT", bufs=1, space="PSUM") as psT, \
                     tc.tile_pool(name="psS", bufs=2, space="PSUM") as psS, \
                     tc.tile_pool(name="psU", bufs=1, space="PSUM") as psU:
    # chunk-0 k-path inputs first so the tensor engine starts early;
                    # the rest of the weight chunks stream behind xb so
                    # head-pair co's inputs land just in time
                    wq_sb = p1w.tile([128, DC, D], f8, tag="wproj")
                    wk_sb = p1w.tile([128, DC, D], f8, tag="wproj_k")
                    xb = p1.tile([128, 4, DC, 512], f8, tag="xb")
                    xqb = p1.tile([128, DC, LQ], f8, tag="xqb")
                    nc.sync.dma_start(xqb, xqb_v)
                    nc.sync.dma_start(wq_sb[:, 0, :], wq_d[0])
                    nc.sync.dma_start(wk_sb[:, 0, :], wk_d[0])
                    for mt in range(4):
                        nc.sync.dma_start(
                            xb[:, mt, :, :],
                            xb_d[mt].rearrange("(c p) m -> p c m", p=128))
                    ident = consts.tile([128, 128], bf16, tag="ident")
                    nc.sync.dma_start(ident, ident_d)
                    for co in range(1, DC):
                        nc.sync.dma_start(wq_sb[:, co, :], wq_d[co])
                        nc.sync.dma_start(wk_sb[:, co, :], wk_d[co])

    # constants (small DMAs, off the critical path)
                    ones_bf = consts.tile([128, 1], bf16, tag="ones")
                    nc.vector.memset(ones_bf, 1.0)
                    # selector rows: sel<s> broadcasts a [1,512] row onto
                    # partitions s*64..s*64+64 via a PE matmul
                    sel0 = consts.tile([1, 128], bf16, tag="sel0")
                    nc.vector.memset(sel0, 0.0)
                    nc.vector.memset(sel0[:, 0:64], 1.0)
                    sel1 = consts.tile([1, 128], bf16, tag="sel1")
                    nc.vector.memset(sel1, 0.0)
                    nc.vector.memset(sel1[:, 64:128], 1.0)
                    ones_row = consts.tile([1, 128], f32, tag="ones_row")
                    nc.vector.memset(ones_row, 1.0)
                    eps_t = consts.tile([1, 1], f32, tag="eps")
                    nc.vector.memset(eps_t, EPS)
                    bb1_sb = consts.tile([128, FC], f32, tag="bb1")
                    nc.sync.dma_start(bb1_sb, bb1_v)
                    bb2_sb = consts.tile([128, DC], f32, tag="bb2")
                    nc.sync.dma_start(bb2_sb, bb2_v)
                    g1_sb = consts.tile([128, DC], f32, tag="g1")
                    nc.sync.dma_start(g1_sb, g1_v)
                    b1_sb = consts.tile([128, DC], f32, tag="b1")
                    nc.sync.dma_start(b1_sb, b1_v)
                    g2_sb = consts.tile([128, DC], f32, tag="g2")
                    nc.sync.dma_start(g2_sb, g2_v)
                    b2_sb = consts.tile([128, DC], f32, tag="b2")
                    nc.sync.dma_start(b2_sb, b2_v)
                    kaug_b = kaug.rearrange("p mj h (two f) -> p mj (h two) f",
                                            two=2)
                    # ones column = 8.0: wk is host-scaled by 8, so kT holds
                    # 8*k; den row becomes 8*sum(e), cancelling the 8 in ctx
                    nc.vector.memset(kaug_b[:, :, :, 64:65], 8.0)
                    nc.vector.memset(kaug_b[:, :, :, 65:128], 0.0)

                    wo_sb = wop.tile([128, DC, D], f8, tag="wo_sb")
                    scd = dramsc.tile([NH, LQ], bf16, tag="rec_sc")

                    for co in range(DC):
                        if co == 2:
                            # prefetch wo once the input stream has drained
                            for cw in range(DC):
                                nc.sync.dma_start(wo_sb[:, cw, :], wo_d[cw])
                        # ---- q chunk co ----
                        psq = psK.tile([128, 512], f32, tag="psk")
                        for cp in range(DC // 2):
                            nc.tensor.matmul(
                                psq,
                                wq_sb[:, co, cp * 256:(cp + 1) * 256]
                                .rearrange("p (two f) -> p two f", two=2),
                                xqb[:, 2 * cp:2 * cp + 2, :],
                                start=(cp == 0), stop=(cp == DC // 2 - 1),
                                perf_mode=DR)
                        nc.vector.tensor_copy(qT[:, co, :], psq)

                        # ---- k chunk co over full L ----
                        for mt in range(4):
                            ps = psK.tile([128, 512], f32, tag="psk")
                            for cp in range(DC // 2):
                                nc.tensor.matmul(
                                    ps,
                                    wk_sb[:, co, cp * 256:(cp + 1) * 256]
                                    .rearrange("p (two f) -> p two f", two=2),
                                    xb[:, mt, 2 * cp:2 * cp + 2, :],
                                    start=(cp == 0), stop=(cp == DC // 2 - 1),
                                    perf_mode=DR)
                            nc.vector.tensor_copy(
                                kT[:, co, mt * 512:(mt + 1) * 512], ps)

                        # ---- transposes -> kaug for heads 2co, 2co+1 ----
                        for g in range(2):
                            pt = psT.tile([128, 1024], bf16, tag="pt")
                            for j in range(8):
                                mi = g * 8 + j
                                nc.tensor.transpose(
                                    pt[:, j * 128:(j + 1) * 128],
                                    kT[:, co, mi * 128:(mi + 1) * 128], ident)
                            ptv = pt.rearrange("p (m he) -> p m he", he=128)
                            for s in range(2):
                                for i in range(2):
                                    # m-chunks g*8+i, g*8+i+2, ... (parity i)
                                    nc.vector.tensor_copy(
                                        kaug[:, g * 4:(g + 1) * 4, 2 * co + s,
                                             i * 128:i * 128 + 64],
                                        ptv[:, i::2, s * 64:(s + 1) * 64])

                        # ---- heads 2co, 2co+1 ----
                        cT = cpool.tile([128, LQ], bf16, tag="cT")
                        den_bc = cpool.tile([128, LQ], bf16, tag="den_bc")
                        rec_pair = []
                        for s in range(2):
                            h = 2 * co + s
                            poff = 64 * s
                            e = epool.tile([128, MC, LQ], f8, tag="E")
                            for mt in range(MC // 2):
                                st = psS.tile([128, 1024], f32, tag="st")
                                for j in range(2):
                                    mi = mt * 2 + j
                                    nc.tensor.matmul(
                                        st[:, j * 512:(j + 1) * 512],
                                        kT[poff:poff + 64, co,
                                           mi * 128:(mi + 1) * 128],
                                        qT[poff:poff + 64, co, :],
                                        start=True, stop=True)
                                # wq,wk host-scaled by 8 => scores are 64x;
                                # fold 1/sqrt(HD)/64 = 1/512 into the exp
                                nc.scalar.activation(
                                    e[:, mt * 2:(mt + 1) * 2, :]
                                    .rearrange("p a b -> p (a b)"),
                                    st, AF.Exp, scale=1.0 / 512.0)
                            u = psU.tile([128, 512], f32, tag="u")
                            for mj in range(MC // 2):
                                nc.tensor.matmul(
                                    u, kaug[:, mj, h, :]
                                    .rearrange("p (two f) -> p two f", two=2),
                                    e[:, 2 * mj:2 * mj + 2, :],
                                    start=(mj == 0), stop=(mj == MC // 2 - 1),
                                    perf_mode=DR)
                            nc.vector.tensor_copy(cT[poff:poff + 64, :],
                                                  u[0:64, :])
                            drow = sm2.tile([1, LQ], f32, tag="drow")
                            nc.vector.tensor_copy(drow, u[64:65, :])
                            rec32 = sm2.tile([1, LQ], f32, tag="rec32")
                            nc.vector.reciprocal_approx_fast(rec32, drow)
                            rec16 = sm2.tile([1, LQ], bf16, tag="rec16")
                            nc.vector.tensor_copy(rec16, rec32)
                            if co < DC - 1:
                                nc.sync.dma_start(scd[h:h + 1, :], rec16)
                                nc.sync.dma_start(
                                    den_bc[poff:poff + 64, :],
                                    scd[h:h + 1, :].partition_broadcast(64))
                            else:
                                # last pair: PE selector broadcast (psK is
                                # free) -- keeps the DMA round-trip off the
                                # critical path into the wo loop
                                rec_pair.append(rec16)
                        if co < DC - 1:
                            nc.vector.tensor_tensor(ctxT[:, co, :], cT, den_bc,
                                                    OP.mult)
                        else:
                            den_ps = psK.tile([128, LQ], f32, tag="psk")
                            nc.tensor.matmul(den_ps, sel0, rec_pair[0],
                                             start=True, stop=False)
                            nc.tensor.matmul(den_ps, sel1, rec_pair[1],
                                             start=False, stop=True)
                            nc.vector.tensor_tensor(ctxT[:, co, :], cT, den_ps,
                                                    OP.mult)

                # ---- attn_out + residual -> r1T, with LN1 prep folded in ----
                with tc.tile_pool(name="r1p", bufs=1) as r1p, \
                     tc.tile_pool(name="psL1", bufs=1, space="PSUM") as psL1, \
                     tc.tile_pool(name="cen1p", bufs=2) as cen1p, \
                     tc.tile_pool(name="psM1", bufs=1, space="PSUM") as psM1:
                    s1_ps = psL1.tile([1, LQ], f32, tag="ln1_sum_r")
                    q1_ps = psL1.tile([1, LQ], f32, tag="ln1_sum_s")
                    r1T = r1p.tile([128, DC, LQ], f32, tag="r1T")
                    r1B = r1p.tile([128, DC, LQ], bf16, tag="r1B")
                    xq_all = r1p.tile([128, DC, LQ], f32, tag="xq_all")
                    for cw in range(DC):
                        nc.sync.dma_start(xq_all[:, cw, :], xq_v[:, cw, :])
                    with tc.tile_pool(name="psB", bufs=4, space="PSUM") as psB:
                        for f in range(DC):
                            ps = psB.tile([128, 512], f32, tag="ao")
                            for cp in range(DC // 2):
                                nc.tensor.matmul(
                                    ps,
                                    wo_sb[:, f, cp * 256:(cp + 1) * 256]
                                    .rearrange("p (two f) -> p two f", two=2),
                                    ctxT[:, 2 * cp:2 * cp + 2, :],
                                    start=(cp == 0), stop=(cp == DC // 2 - 1),
                                    perf_mode=DR)
                            nc.vector.tensor_tensor(r1T[:, f, :], ps,
                                                    xq_all[:, f, :], OP.add)
                            rb1 = r1B[:, f, :]
                            nc.vector.tensor_copy(rb1, r1T[:, f, :])
                            sq1 = sm2.tile([128, 512], bf16, tag="sq1")
                            nc.vector.tensor_tensor(sq1, rb1, rb1, OP.mult)
                            nc.tensor.matmul(s1_ps, ones_bf, rb1,
                                             start=(f == 0), stop=(f == DC - 1))
                            nc.tensor.matmul(q1_ps, ones_bf, sq1,
                                             start=(f == 0), stop=(f == DC - 1))

                    # prefetch the first w1 tiles so the FFN stream starts
                    # as soon as hb chunks appear
                    w1_pre = []
                    for i in range(2):
                        w1t = sm2.tile([128, D], f8, tag="w1pre")
                        nc.sync.dma_start(w1t, w1_d[i])
                        w1_pre.append(w1t)

                    # ---- LN1 stats + normalize (chunkwise) -> hT, hb ----
                    mu = sm.tile([1, LQ], f32, tag="ln_mu")
                    nc.scalar.activation(mu, s1_ps, AF.Copy, scale=1.0 / D)
                    msq = sm.tile([1, LQ], f32, tag="ln_msq")
                    nc.scalar.activation(msq, q1_ps, AF.Copy, scale=1.0 / D)
                    var = sm.tile([1, LQ], f32, tag="ln_var")
                    nc.vector.tensor_tensor(var, mu, mu, OP.mult)
                    nc.vector.tensor_tensor(var, msq, var, OP.subtract)
                    std = sm.tile([1, LQ], f32, tag="ln_std")
                    nc.scalar.activation(std, var, AF.Sqrt, bias=eps_t)
                    mrrow = sm.tile([1, 2 * LQ], f32, tag="ln_mrrow")
                    nc.vector.reciprocal_approx_fast(mrrow[:, LQ:2 * LQ], std)
                    nc.vector.tensor_copy(mrrow[:, 0:LQ], mu)
                    mr_ps = psM1.tile([128, 2 * LQ], f32, tag="ln_mrps")
                    for j in range(2):
                        nc.tensor.matmul(mr_ps[:, j * LQ:(j + 1) * LQ], ones_row,
                                         mrrow[:, j * LQ:(j + 1) * LQ],
                                         start=True, stop=True)
                    mu_bc, rstd_bc = mr_ps[:, 0:LQ], mr_ps[:, LQ:2 * LQ]
                    # hT holds raw cen = (r1-mu)*rstd (bf16); g1/b1 fold into
                    # w1 on the host (w1g, bb1') for the z path, and apply
                    # via an ACT affine inside the FFN window for the residual.
                    # Normalize runs at bf16 DVE width from the r1B copy.
                    mr16 = cen1p.tile([128, 2 * LQ], bf16, tag="mr16")
                    nc.vector.tensor_copy(mr16, mr_ps)
                    for c in range(DC):
                        cen = cen1p.tile([128, LQ], bf16, tag="ln_cen")
                        nc.vector.tensor_tensor(cen, r1B[:, c, :],
                                                mr16[:, 0:LQ], OP.subtract)
                        nc.vector.tensor_tensor(hT[:, c, :], cen,
                                                mr16[:, LQ:2 * LQ], OP.mult)
                        nc.scalar.activation(hb[:, c, :], hT[:, c, :], AF.Copy)
            # ---- Phase 3: FFN ----
            with tc.tile_pool(name="ffn", bufs=1) as ffn, \
                 tc.tile_pool(name="w1stream", bufs=4) as w1stream, \
                 tc.tile_pool(name="w2pool", bufs=1) as w2pool:
                g_sb = ffn.tile([128, FC, LQ], f8, tag="g")
                r2T = ffn.tile([128, DC, LQ], f32, tag="r2T")
                r2B = ffn.tile([128, DC, LQ], bf16, tag="r2B")
                w2_sb = w2pool.tile([128, FC // 2, 2 * D], f8, tag="w2_sb")

                with tc.tile_pool(name="psL2", bufs=1, space="PSUM") as psL2:
                  s2_ps = psL2.tile([1, LQ], f32, tag="ln2_sum_r")
                  q2_ps = psL2.tile([1, LQ], f32, tag="ln2_sum_s")
                  with tc.tile_pool(name="psZO", bufs=1, space="PSUM") as psZO:
                    for half in range(2):
                        o_ps = [psZO.tile([128, 512], f32, tag=f"o{f}", name=f"o_ps{f}")
                                for f in range(4)]
                        for j in range(FC // 2):
                            if half == 0:
                                for i in (2 * j, 2 * j + 1):
                                    if i < 2:
                                        w1t = w1_pre[i]
                                    else:
                                        w1t = w1stream.tile([128, D], f8, tag="w1t")
                                        nc.sync.dma_start(w1t, w1_d[i])
                                    zt = psZO.tile([128, 512], f32, tag=f"zt{i % 2}",
                                                   name=f"zt{i % 2}")
                                    for cp in range(DC // 2):
                                        nc.tensor.matmul(
                                            zt,
                                            w1t[:, cp * 256:(cp + 1) * 256]
                                            .rearrange("p (two f) -> p two f", two=2),
                                            hb[:, 2 * cp:2 * cp + 2, :],
                                            start=(cp == 0), stop=(cp == DC // 2 - 1),
                                            perf_mode=DR)
                                    # w1 host-scaled by 8: z = zt/8 + bb1
                                    nc.scalar.activation(g_sb[:, i, :], zt, AF.Gelu,
                                                         scale=0.125,
                                                         bias=bb1_sb[:, i:i + 1])
                                nc.sync.dma_start(w2_sb[:, j, :], w2_d[j])
                            for f in range(4):
                                fo = half * 4 + f
                                nc.tensor.matmul(
                                    o_ps[f],
                                    w2_sb[:, j, fo * 256:(fo + 1) * 256]
                                    .rearrange("p (two f) -> p two f", two=2),
                                    g_sb[:, 2 * j:2 * j + 2, :],
                                    start=(j == 0), stop=(j == FC // 2 - 1),
                                    perf_mode=DR)
                        for f in range(4):
                            fo = half * 4 + f
                            t = sm2.tile([128, 512], f32, tag="obias")
                            # w2 host-scaled by 16: o = o_ps/16 + bb2
                            nc.scalar.activation(t, o_ps[f], AF.Identity,
                                                 scale=0.0625,
                                                 bias=bb2_sb[:, fo:fo + 1])
                            # h = g1*cen + b1 (deferred LN1 affine)
                            h_aff = sm2.tile([128, 512], f32, tag="h_aff")
                            nc.scalar.activation(h_aff, hT[:, fo, :], AF.Identity,
                                                 scale=g1_sb[:, fo:fo + 1],
                                                 bias=b1_sb[:, fo:fo + 1])
                            nc.vector.tensor_tensor(r2T[:, fo, :], t, h_aff, OP.add)
                            # LN2 prep folded in: bf16 copy + square + partial sums
                            rb2 = r2B[:, fo, :]
                            nc.vector.tensor_copy(rb2, r2T[:, fo, :])
                            sq2 = sm2.tile([128, 512], bf16, tag="sq2")
                            nc.vector.tensor_tensor(sq2, rb2, rb2, OP.mult)
                            nc.tensor.matmul(s2_ps, ones_bf, rb2,
                                             start=(fo == 0), stop=(fo == D // 128 - 1))
                            nc.tensor.matmul(q2_ps, ones_bf, sq2,
                                             start=(fo == 0), stop=(fo == D // 128 - 1))

                  # ---- LN2 stats + normalize -> out (chunked DMA) ----
                  with tc.tile_pool(name="ln2out", bufs=3) as ln2out, \
                       tc.tile_pool(name="psM2", bufs=1, space="PSUM") as psM2:
                      mu = sm.tile([1, LQ], f32, tag="ln_mu")
                      nc.scalar.activation(mu, s2_ps, AF.Copy, scale=1.0 / D)
                      msq = sm.tile([1, LQ], f32, tag="ln_msq")
                      nc.scalar.activation(msq, q2_ps, AF.Copy, scale=1.0 / D)
                      var = sm.tile([1, LQ], f32, tag="ln_var")
                      nc.vector.tensor_tensor(var, mu, mu, OP.mult)
                      nc.vector.tensor_tensor(var, msq, var, OP.subtract)
                      std = sm.tile([1, LQ], f32, tag="ln_std")
                      nc.scalar.activation(std, var, AF.Sqrt, bias=eps_t)
                      mrrow = sm.tile([1, 2 * LQ], f32, tag="ln_mrrow")
                      nc.vector.reciprocal_approx_fast(mrrow[:, LQ:2 * LQ], std)
                      nc.vector.tensor_copy(mrrow[:, 0:LQ], mu)
                      mr_ps = psM2.tile([128, 2 * LQ], f32, tag="ln_mrps")
                      for j in range(2):
                          nc.tensor.matmul(mr_ps[:, j * LQ:(j + 1) * LQ], ones_row,
                                           mrrow[:, j * LQ:(j + 1) * LQ],
                                           start=True, stop=True)
                      mr16b = ln2out.tile([128, 2 * LQ], bf16, tag="mr16b")
                      nc.vector.tensor_copy(mr16b, mr_ps)
                      for c in range(DC):
                          cen = ln2out.tile([128, LQ], bf16, tag="ln_cen")
                          nc.vector.tensor_tensor(cen, r2B[:, c, :],
                                                  mr16b[:, 0:LQ], OP.subtract)
                          nc.vector.tensor_tensor(cen, cen,
                                                  mr16b[:, LQ:2 * LQ], OP.mult)
                          oc = ln2out.tile([128, LQ], f32, tag="ln_oc")
                          nc.scalar.activation(oc, cen, AF.Identity,
                                               scale=g2_sb[:, c:c + 1], bias=b2_sb[:, c:c + 1])
                          nc.sync.dma_start(out_v[:, c, :], oc)

    nc.compile()
    return nc


def _get_nc():
    if "nc" not in _cache:
        _cache["nc"] = _build_nc()
    return _cache["nc"]


def _host_prep(inputs):
    x = np.asarray(inputs["x"], np.float32)
    wq = np.asarray(inputs["wq"], np.float32)
    wk = np.asarray(inputs["wk"], np.float32)
    wo = np.asarray(inputs["wo"], np.float32)
    g1 = np.asarray(inputs["g1"], np.float32)
    b1 = np.asarray(inputs["b1"], np.float32)
    w1 = np.asarray(inputs["w1"], np.float32)
    bb1 = np.asarray(inputs["bb1"], np.float32)
    w2 = np.asarray(inputs["w2"], np.float32)
    bb2 = np.asarray(inputs["bb2"], np.float32)
    g2 = np.asarray(inputs["g2"], np.float32)
    b2 = np.asarray(inputs["b2"], np.float32)

    idx = np.arange(D)
    perm = (idx % HD) * NH + (idx // HD)  # f' = h*64+d  ->  old f = d*16+h

    def bf(a):
        return np.ascontiguousarray(a).astype(BF16NP)

    def f8(a):
        return np.ascontiguousarray(a).astype(F8NP)

    # w1 x8 / w2 x16 / wq,wk x8: power-of-2 pre-scales keep fp8 weights out of
    # the subnormal range; compensated on-device (gelu/obias/exp scale args,
    # kaug ones column = 8).  LN1's gamma/beta fold into w1/bb1 (z = cen@w1g
    # + bb1') so the device feeds raw cen into the FFN.
    w1g = w1 * g1[:, None]
    bb1 = bb1 + b1 @ w1
    w1t = (8.0 * w1g).reshape(DC, 128, FC, 128).transpose(2, 1, 0, 3).reshape(FC, 128, D)
    # w2p[j, p, fc*256 + i*128 + c] = 16*w2[(2j+i)*128+p, fc*128+c]
    w2p = (16.0 * w2).reshape(FC // 2, 2, 128, DC, 128).transpose(0, 2, 3, 1, 4) \
        .reshape(FC // 2, 128, 2 * D)
    def coblock(a):  # [K, F] -> [F-chunk, K-part, K-chunk-major cols]
        return a.reshape(DC, 128, DC, 128).transpose(2, 1, 0, 3).reshape(DC, 128, D)

    shared = {
        "wq": f8(coblock(8.0 * wq[:, perm])),
        "wk": f8(coblock(8.0 * wk[:, perm])),
        "wo": f8(coblock(wo[perm, :])),
        "w1": f8(w1t), "w2": f8(w2p),
        "ident": bf(np.eye(128, dtype=np.float32)),
        "bb1": bb1, "bb2": bb2, "g1": g1, "b1": b1, "g2": g2, "b2": b2,
    }
    in_maps = []
    for c in range(NCORES):
        b, q0 = c // (NCORES // B), (c % (NCORES // B)) * LQ
        xT = np.ascontiguousarray(x[b].T)
        m = dict(shared)
        m["xb"] = f8(np.ascontiguousarray(
            xT.reshape(D, 4, 512).transpose(1, 0, 2)))
        m["xqb"] = f8(xT[:, q0:q0 + LQ])
        m["xq"] = np.ascontiguousarray(xT[:, q0:q0 + LQ])
        in_maps.append(m)
    return in_maps


def kernel(**inputs):
    global LAST_RESULTS
    from concourse.bass_utils import run_bass_kernel_spmd

    nc = _get_nc()
    in_maps = _host_prep(inputs)
    res = run_bass_kernel_spmd(nc, in_maps, core_ids=list(range(NCORES)))
    LAST_RESULTS = res
    out = np.empty((B, L, D), np.float32)
    for c in range(NCORES):
        b, q0 = c // (NCORES // B), (c % (NCORES // B)) * LQ
        out[b, q0:q0 + LQ, :] = res.results[c]["out"].T
    return out


# revision 51
# speedup vs baseline: 1.0873x; 1.0532x over previous
"""Trainium2 Bass kernel for nn_EncoderLayer (B=2, L=2048, D=1024, 16 heads, FFN 4096).

Strategy: sequence-parallel over the 8 cores (core c owns batch c//4, query rows
(c%4)*512 .. +512).  Each core recomputes the full K projection for its batch,
which avoids all collectives; everything else is local.

v4 (485us -> ~340us HW): fp8 (e4m3) DoubleRow matmuls for the q/k projections,
attn@K, wo, w1 AND w2 — 2x PE throughput (256-wide contraction per 512-cycle
instruction; verified on hw: same 216ns issue rate as a 128-wide bf16 matmul).
Scores stay bf16 (output-column-bound, fp8 gives no gain).  DoubleRow
stationaries must be contiguous [p,256] (walrus ISA check), hence the
co-blocked wq/wk/wo host layouts, the [128, MC/2, NH, 256] kaug layout with
m-chunk pairs adjacent per head, and the pair-interleaved w2 layout.

Precision plan (measured rel err ~1.0e-2 vs 2e-2 gate, matches numpy sim):
weights pre-scaled by powers of 2 (wq,wk,w1 x8; w2 x16) to avoid fp8
subnormals, compensated for free in ACT scale args (exp 1/512, gelu 1/8,
obias 1/16) and the kaug ones-column (=8, cancelling wk's x8 in the softmax
denominator).  Residual/LN paths stay fp32; attention fp8 error is diluted
~100x because the (faithfully reproduced) attn@K-instead-of-V bug makes
attn_out ~1% of x.

Schedule: per-head-pair pipeline (K-proj chunk co -> PE transposes -> scores/
exp/attn@K for heads 2co,2co+1) so ACT exp (~143us total, the attention-phase
floor) fills from ~6us in; PE projection work hides in ACT-bound slack.
Softmax denominators: per-pair reciprocal + DRAM partition-broadcast hidden
under the next pair's compute; the last pair uses a PE selector-matmul
broadcast (psK is free) to keep the DMA round-trip off the critical path into
wo.  LN1's gamma/beta fold into w1/bb1 on the host, so the LN1->FFN seam only
needs sub/mult/fp8-cast per chunk; the residual affine runs inside the FFN
window where ACT is idle.  Both LN normalizes run at bf16 DVE width from the
bf16 copies already made for the stats sums (r1B/r2B) — this closed the LN1
seam to ~2us and costs ~0.1% extra rel err.  x-residual and first w1 tiles
prefetch during attention/wo.  Remaining known costs: ~14us LN2 tail (ACT
affine + serial stats chain + output DMA drain), ~16us startup, DVFS
throttling (util limit ~0.72-0.78 under fp8 load; run-to-run noise +-10us).
"""

import sys
sys.setrecursionlimit(200000)
import numpy as np
import ml_dtypes

B, L, D, NH, HD, FF = 2, 2048, 1024, 16, 64, 4096
LQ = 512  # query rows per core
NCORES = 8
EPS = 1e-5
DC = D // 128  # 8 feature chunks
MC = L // 128  # 16 key chunks
FC = FF // 128  # 32 ffn chunks
BF16NP = ml_dtypes.bfloat16
F8NP = ml_dtypes.float8_e4m3

_cache = {}
LAST_RESULTS = None


def _build_nc():
    import concourse.bass as bass
    import concourse.tile as tile
    from concourse import bacc, mybir
    from contextlib import ExitStack

    f32 = mybir.dt.float32
    bf16 = mybir.dt.bfloat16
    f8 = mybir.dt.float8e4
    AF = mybir.ActivationFunctionType
    OP = mybir.AluOpType
    DR = mybir.MatmulPerfMode.DoubleRow

    nc = bacc.Bacc("TRN2", debug=False, target_bir_lowering=False)

    # ---- DRAM I/O ----
    xb_d = nc.dram_tensor("xb", [4, D, 512], f8, kind="ExternalInput").ap()
    xqb_d = nc.dram_tensor("xqb", [D, LQ], f8, kind="ExternalInput").ap()
    xq_d = nc.dram_tensor("xq", [D, LQ], f32, kind="ExternalInput").ap()
    wq_d = nc.dram_tensor("wq", [DC, 128, D], f8, kind="ExternalInput").ap()
    wk_d = nc.dram_tensor("wk", [DC, 128, D], f8, kind="ExternalInput").ap()
    wo_d = nc.dram_tensor("wo", [DC, 128, D], f8, kind="ExternalInput").ap()
    w1_d = nc.dram_tensor("w1", [FC, 128, D], f8, kind="ExternalInput").ap()
    w2_d = nc.dram_tensor("w2", [FC // 2, 128, 2 * D], f8, kind="ExternalInput").ap()
    ident_d = nc.dram_tensor("ident", [128, 128], bf16, kind="ExternalInput").ap()
    bb1_d = nc.dram_tensor("bb1", [FF], f32, kind="ExternalInput").ap()
    bb2_d = nc.dram_tensor("bb2", [D], f32, kind="ExternalInput").ap()
    g1_d = nc.dram_tensor("g1", [D], f32, kind="ExternalInput").ap()
    b1_d = nc.dram_tensor("b1", [D], f32, kind="ExternalInput").ap()
    g2_d = nc.dram_tensor("g2", [D], f32, kind="ExternalInput").ap()
    b2_d = nc.dram_tensor("b2", [D], f32, kind="ExternalInput").ap()
    out_d = nc.dram_tensor("out", [D, LQ], f32, kind="ExternalOutput").ap()

    xqb_v = xqb_d.rearrange("(c p) l -> p c l", p=128)
    xq_v = xq_d.rearrange("(c p) l -> p c l", p=128)
    bb1_v = bb1_d.rearrange("(c p) -> p c", p=128)
    bb2_v = bb2_d.rearrange("(c p) -> p c", p=128)
    g1_v = g1_d.rearrange("(c p) -> p c", p=128)
    b1_v = b1_d.rearrange("(c p) -> p c", p=128)
    g2_v = g2_d.rearrange("(c p) -> p c", p=128)
    b2_v = b2_d.rearrange("(c p) -> p c", p=128)
    out_v = out_d.rearrange("(c p) l -> p c l", p=128)

    with tile.TileContext(nc, pool_alloc_mode="queue") as tc, ExitStack() as top:
        consts = top.enter_context(tc.tile_pool(name="consts", bufs=1))
        dramsc = top.enter_context(tc.tile_pool(name="dramsc", bufs=2, space="DRAM"))

        sm = top.enter_context(tc.tile_pool(name="smalls", bufs=1))
        sm2 = top.enter_context(tc.tile_pool(name="smalls2", bufs=2))

        with tc.tile_pool(name="mid", bufs=1) as mid:
            hT = mid.tile([128, DC, LQ], bf16, tag="hT")
            hb = mid.tile([128, DC, LQ], f8, tag="hb")

            with tc.tile_pool(name="kq", bufs=1) as kq:
                kT = kq.tile([128, DC, L], bf16, tag="kT")
                # kaug[p, mj, h, i*128 + j]: m-chunk pair mj, head h, k-tile i
                # (m = 2*mj+i), col j in [0:64] = head dims, 64 = ones, rest pad
                kaug = kq.tile([128, MC // 2, NH, 256], f8, tag="kaug")
                qT = kq.tile([128, DC, LQ], bf16, tag="qT")
                ctxT = kq.tile([128, DC, LQ], f8, tag="ctxT")

                # ---- Phase 1+2: interleaved projections + attention ----
                with tc.tile_pool(name="p1", bufs=1) as p1, \
                     tc.tile_pool(name="p1w", bufs=1) as p1w, \
                     tc.tile_pool(name="epool", bufs=2) as epool, \
                     tc.tile_pool(name="cpool", bufs=2) as cpool, \
                     tc.tile_pool(name="wop", bufs=1) as wop, \
                     tc.tile_pool(name="psK", bufs=2, space="PSUM") as psK, \
                     tc.tile_pool(name="psT", bufs=1, space="PSUM") as psT, \
                     tc.tile_pool(name="psS", bufs=2, space="PSUM") as psS, \
                     tc.tile_pool(name="psU", bufs=1, space="PSUM") as psU:
    # chunk-0 k-path inputs first so the tensor engine starts early;
                    # the rest of the weight chunks stream behind xb so
                    # head-pair co's inputs land just in time
                    wq_sb = p1w.tile([128, DC, D], f8, tag="wproj")
                    wk_sb = p1w.tile([128, DC, D], f8, tag="wproj_k")
                    xb = p1.tile([128, 4, DC, 512], f8, tag="xb")
                    xqb = p1.tile([128, DC, LQ], f8, tag="xqb")
                    nc.sync.dma_start(xqb, xqb_v)
                    nc.sync.dma_start(wq_sb[:, 0, :], wq_d[0])
                    nc.sync.dma_start(wk_sb[:, 0, :], wk_d[0])
                    for mt in range(4):
                        nc.sync.dma_start(
                            xb[:, mt, :, :],
                            xb_d[mt].rearrange("(c p) m -> p c m", p=128))
                    ident = consts.tile([128, 128], bf16, tag="ident")
                    nc.sync.dma_start(ident, ident_d)
                    for co in range(1, DC):
                        nc.sync.dma_start(wq_sb[:, co, :], wq_d[co])
                        nc.sync.dma_start(wk_sb[:, co, :], wk_d[co])

    # constants (small DMAs, off the critical path)
                    ones_bf = consts.tile([128, 1], bf16, tag="ones")
                    nc.vector.memset(ones_bf, 1.0)
                    # selector rows: sel<s> broadcasts a [1,512] row onto
                    # partitions s*64..s*64+64 via a PE matmul
                    sel0 = consts.tile([1, 128], bf16, tag="sel0")
                    nc.vector.memset(sel0, 0.0)
                    nc.vector.memset(sel0[:, 0:64], 1.0)
                    sel1 = consts.tile([1, 128], bf16, tag="sel1")
                    nc.vector.memset(sel1, 0.0)
                    nc.vector.memset(sel1[:, 64:128], 1.0)
                    ones_row = consts.tile([1, 128], f32, tag="ones_row")
                    nc.vector.memset(ones_row, 1.0)
                    eps_t = consts.tile([1, 1], f32, tag="eps")
                    nc.vector.memset(eps_t, EPS)
                    bb1_sb = consts.tile([128, FC], f32, tag="bb1")
                    nc.sync.dma_start(bb1_sb, bb1_v)
                    bb2_sb = consts.tile([128, DC], f32, tag="bb2")
                    nc.sync.dma_start(bb2_sb, bb2_v)
                    g1_sb = consts.tile([128, DC], f32, tag="g1")
                    nc.sync.dma_start(g1_sb, g1_v)
                    b1_sb = consts.tile([128, DC], f32, tag="b1")
                    nc.sync.dma_start(b1_sb, b1_v)
                    g2_sb = consts.tile([128, DC], f32, tag="g2")
                    nc.sync.dma_start(g2_sb, g2_v)
                    b2_sb = consts.tile([128, DC], f32, tag="b2")
                    nc.sync.dma_start(b2_sb, b2_v)
                    kaug_b = kaug.rearrange("p mj h (two f) -> p mj (h two) f",
                                            two=2)
                    # ones column = 8.0: wk is host-scaled by 8, so kT holds
                    # 8*k; den row becomes 8*sum(e), cancelling the 8 in ctx
                    nc.vector.memset(kaug_b[:, :, :, 64:65], 8.0)

                    wo_sb = wop.tile([128, DC, D], f8, tag="wo_sb")
                    scd = dramsc.tile([NH, LQ], bf16, tag="rec_sc")

                    for co in range(DC):
                        if co == 2:
                            # prefetch wo once the input stream has drained
                            for cw in range(DC):
                                nc.sync.dma_start(wo_sb[:, cw, :], wo_d[cw])
                        # ---- q chunk co ----
                        psq = psK.tile([128, 512], f32, tag="psk")
                        for cp in range(DC // 2):
                            nc.tensor.matmul(
                                psq,
                                wq_sb[:, co, cp * 256:(cp + 1) * 256]
                                .rearrange("p (two f) -> p two f", two=2),
                                xqb[:, 2 * cp:2 * cp + 2, :],
                                start=(cp == 0), stop=(cp == DC // 2 - 1),
                                perf_mode=DR)
                        nc.vector.tensor_copy(qT[:, co, :], psq)

                        # ---- k chunk co over full L ----
                        for mt in range(4):
                            ps = psK.tile([128, 512], f32, tag="psk")
                            for cp in range(DC // 2):
                                nc.tensor.matmul(
                                    ps,
                                    wk_sb[:, co, cp * 256:(cp + 1) * 256]
                                    .rearrange("p (two f) -> p two f", two=2),
                                    xb[:, mt, 2 * cp:2 * cp + 2, :],
                                    start=(cp == 0), stop=(cp == DC // 2 - 1),
                                    perf_mode=DR)
                            nc.vector.tensor_copy(
                                kT[:, co, mt * 512:(mt + 1) * 512], ps)

                        # ---- transposes -> kaug for heads 2co, 2co+1 ----
                        for g in range(2):
                            pt = psT.tile([128, 1024], bf16, tag="pt")
                            for j in range(8):
                                mi = g * 8 + j
                                nc.tensor.transpose(
                                    pt[:, j * 128:(j + 1) * 128],
                                    kT[:, co, mi * 128:(mi + 1) * 128], ident)
                            ptv = pt.rearrange("p (m he) -> p m he", he=128)
                            for s in range(2):
                                for i in range(2):
                                    # m-chunks g*8+i, g*8+i+2, ... (parity i)
                                    nc.vector.tensor_copy(
                                        kaug[:, g * 4:(g + 1) * 4, 2 * co + s,
                                             i * 128:i * 128 + 64],
                                        ptv[:, i::2, s * 64:(s + 1) * 64])
                        # zero this pair's kaug padding off the critical DVE
                        # path (one big upfront memset stalled the queue 13us)
                        nc.vector.memset(
                            kaug_b[:, :, 4 * co:4 * co + 4, 65:128], 0.0)

                        # ---- heads 2co, 2co+1 ----
                        cT = cpool.tile([128, LQ], bf16, tag="cT")
                        den_bc = cpool.tile([128, LQ], bf16, tag="den_bc")
                        rec_pair = []
                        for s in range(2):
                            h = 2 * co + s
                            poff = 64 * s
                            e = epool.tile([128, MC, LQ], f8, tag="E")
                            for mt in range(MC // 2):
                                st = psS.tile([128, 1024], f32, tag="st")
                                for j in range(2):
                                    mi = mt * 2 + j
                                    nc.tensor.matmul(
                                        st[:, j * 512:(j + 1) * 512],
                                        kT[poff:poff + 64, co,
                                           mi * 128:(mi + 1) * 128],
                                        qT[poff:poff + 64, co, :],
                                        start=True, stop=True)
                                # wq,wk host-scaled by 8 => scores are 64x;
                                # fold 1/sqrt(HD)/64 = 1/512 into the exp
                                nc.scalar.activation(
                                    e[:, mt * 2:(mt + 1) * 2, :]
                                    .rearrange("p a b -> p (a b)"),
                                    st, AF.Exp, scale=1.0 / 512.0)
                            u = psU.tile([128, 512], f32, tag="u")
                            for mj in range(MC // 2):
                                nc.tensor.matmul(
                                    u, kaug[:, mj, h, :]
                                    .rearrange("p (two f) -> p two f", two=2),
                                    e[:, 2 * mj:2 * mj + 2, :],
                                    start=(mj == 0), stop=(mj == MC // 2 - 1),
                                    perf_mode=DR)
                            nc.vector.tensor_copy(cT[poff:poff + 64, :],
                                                  u[0:64, :])
                            drow = sm2.tile([1, LQ], f32, tag="drow")
                            nc.vector.tensor_copy(drow, u[64:65, :])
                            rec32 = sm2.tile([1, LQ], f32, tag="rec32")
                            nc.vector.reciprocal_approx_fast(rec32, drow)
                            rec16 = sm2.tile([1, LQ], bf16, tag="rec16")
                            nc.vector.tensor_copy(rec16, rec32)
                            if co < DC - 1:
                                nc.sync.dma_start(scd[h:h + 1, :], rec16)
                                nc.sync.dma_start(
                                    den_bc[poff:poff + 64, :],
                                    scd[h:h + 1, :].partition_broadcast(64))
                            else:
                                # last pair: PE selector broadcast (psK is
                                # free) -- keeps the DMA round-trip off the
                                # critical path into the wo loop
                                rec_pair.append(rec16)
                        if co < DC - 1:
                            nc.vector.tensor_tensor(ctxT[:, co, :], cT, den_bc,
                                                    OP.mult)
                        else:
                            den_ps = psK.tile([128, LQ], f32, tag="psk")
                            nc.tensor.matmul(den_ps, sel0, rec_pair[0],
                                             start=True, stop=False)
                            nc.tensor.matmul(den_ps, sel1, rec_pair[1],
                                             start=False, stop=True)
                            nc.vector.tensor_tensor(ctxT[:, co, :], cT, den_ps,
                                                    OP.mult)

                # ---- attn_out + residual -> r1T, with LN1 prep folded in ----
                with tc.tile_pool(name="r1p", bufs=1) as r1p, \
                     tc.tile_pool(name="psL1", bufs=1, space="PSUM") as psL1, \
                     tc.tile_pool(name="cen1p", bufs=2) as cen1p, \
                     tc.tile_pool(name="psM1", bufs=1, space="PSUM") as psM1:
                    s1_ps = psL1.tile([1, LQ], f32, tag="ln1_sum_r")
                    q1_ps = psL1.tile([1, LQ], f32, tag="ln1_sum_s")
                    r1T = r1p.tile([128, DC, LQ], f32, tag="r1T")
                    r1B = r1p.tile([128, DC, LQ], bf16, tag="r1B")
                    xq_all = r1p.tile([128, DC, LQ], f32, tag="xq_all")
                    for cw in range(DC):
                        nc.sync.dma_start(xq_all[:, cw, :], xq_v[:, cw, :])
                    with tc.tile_pool(name="psB", bufs=4, space="PSUM") as psB:
                        for f in range(DC):
                            ps = psB.tile([128, 512], f32, tag="ao")
                            for cp in range(DC // 2):
                                nc.tensor.matmul(
                                    ps,
                                    wo_sb[:, f, cp * 256:(cp + 1) * 256]
                                    .rearrange("p (two f) -> p two f", two=2),
                                    ctxT[:, 2 * cp:2 * cp + 2, :],
                                    start=(cp == 0), stop=(cp == DC // 2 - 1),
                                    perf_mode=DR)
                            nc.vector.tensor_tensor(r1T[:, f, :], ps,
                                                    xq_all[:, f, :], OP.add)
                            rb1 = r1B[:, f, :]
                            nc.vector.tensor_copy(rb1, r1T[:, f, :])
                            sq1 = sm2.tile([128, 512], bf16, tag="sq1")
                            nc.vector.tensor_tensor(sq1, rb1, rb1, OP.mult)
                            nc.tensor.matmul(s1_ps, ones_bf, rb1,
                                             start=(f == 0), stop=(f == DC - 1))
                            nc.tensor.matmul(q1_ps, ones_bf, sq1,
                                             start=(f == 0), stop=(f == DC - 1))

                    # prefetch the first w1 tiles so the FFN stream starts
                    # as soon as hb chunks appear
                    w1_pre = []
                    for i in range(2):
                        w1t = sm2.tile([128, D], f8, tag="w1pre")
                        nc.sync.dma_start(w1t, w1_d[i])
                        w1_pre.append(w1t)

                    # ---- LN1 stats + normalize (chunkwise) -> hT, hb ----
                    mu = sm.tile([1, LQ], f32, tag="ln_mu")
                    nc.scalar.activation(mu, s1_ps, AF.Copy, scale=1.0 / D)
                    msq = sm.tile([1, LQ], f32, tag="ln_msq")
                    nc.scalar.activation(msq, q1_ps, AF.Copy, scale=1.0 / D)
                    var = sm.tile([1, LQ], f32, tag="ln_var")
                    nc.vector.tensor_tensor(var, mu, mu, OP.mult)
                    nc.vector.tensor_tensor(var, msq, var, OP.subtract)
                    std = sm.tile([1, LQ], f32, tag="ln_std")
                    nc.scalar.activation(std, var, AF.Sqrt, bias=eps_t)
                    mrrow = sm.tile([1, 2 * LQ], f32, tag="ln_mrrow")
                    nc.vector.reciprocal_approx_fast(mrrow[:, LQ:2 * LQ], std)
                    nc.vector.tensor_copy(mrrow[:, 0:LQ], mu)
                    mr_ps = psM1.tile([128, 2 * LQ], f32, tag="ln_mrps")
                    for j in range(2):
                        nc.tensor.matmul(mr_ps[:, j * LQ:(j + 1) * LQ], ones_row,
                                         mrrow[:, j * LQ:(j + 1) * LQ],
                                         start=True, stop=True)
                    mu_bc, rstd_bc = mr_ps[:, 0:LQ], mr_ps[:, LQ:2 * LQ]
                    # hT holds raw cen = (r1-mu)*rstd (bf16); g1/b1 fold into
                    # w1 on the host (w1g, bb1') for the z path, and apply
                    # via an ACT affine inside the FFN window for the residual.
                    # Normalize runs at bf16 DVE width from the r1B copy.
                    mr16 = cen1p.tile([128, 2 * LQ], bf16, tag="mr16")
                    nc.vector.tensor_copy(mr16, mr_ps)
                    for c in range(DC):
                        cen = cen1p.tile([128, LQ], bf16, tag="ln_cen")
                        nc.vector.tensor_tensor(cen, r1B[:, c, :],
                                                mr16[:, 0:LQ], OP.subtract)
                        nc.vector.tensor_tensor(hT[:, c, :], cen,
                                                mr16[:, LQ:2 * LQ], OP.mult)
                        nc.scalar.activation(hb[:, c, :], hT[:, c, :], AF.Copy)
            # ---- Phase 3: FFN ----
            with tc.tile_pool(name="ffn", bufs=1) as ffn, \
                 tc.tile_pool(name="w1stream", bufs=4) as w1stream, \
                 tc.tile_pool(name="w2pool", bufs=1) as w2pool:
                g_sb = ffn.tile([128, FC, LQ], f8, tag="g")
                r2T = ffn.tile([128, DC, LQ], f32, tag="r2T")
                r2B = ffn.tile([128, DC, LQ], bf16, tag="r2B")
                w2_sb = w2pool.tile([128, FC // 2, 2 * D], f8, tag="w2_sb")

                with tc.tile_pool(name="psL2", bufs=1, space="PSUM") as psL2:
                  s2_ps = psL2.tile([1, LQ], f32, tag="ln2_sum_r")
                  q2_ps = psL2.tile([1, LQ], f32, tag="ln2_sum_s")
                  with tc.tile_pool(name="psZO", bufs=1, space="PSUM") as psZO:
                    for half in range(2):
                        o_ps = [psZO.tile([128, 512], f32, tag=f"o{f}", name=f"o_ps{f}")
                                for f in range(4)]
                        for j in range(FC // 2):
                            if half == 0:
                                for i in (2 * j, 2 * j + 1):
                                    if i < 2:
                                        w1t = w1_pre[i]
                                    else:
                                        w1t = w1stream.tile([128, D], f8, tag="w1t")
                                        nc.sync.dma_start(w1t, w1_d[i])
                                    zt = psZO.tile([128, 512], f32, tag=f"zt{i % 2}",
                                                   name=f"zt{i % 2}")
                                    for cp in range(DC // 2):
                                        nc.tensor.matmul(
                                            zt,
                                            w1t[:, cp * 256:(cp + 1) * 256]
                                            .rearrange("p (two f) -> p two f", two=2),
                                            hb[:, 2 * cp:2 * cp + 2, :],
                                            start=(cp == 0), stop=(cp == DC // 2 - 1),
                                            perf_mode=DR)
                                    # w1 host-scaled by 8: z = zt/8 + bb1
                                    nc.scalar.activation(g_sb[:, i, :], zt, AF.Gelu,
                                                         scale=0.125,
                                                         bias=bb1_sb[:, i:i + 1])
                                nc.sync.dma_start(w2_sb[:, j, :], w2_d[j])
                            for f in range(4):
                                fo = half * 4 + f
                                nc.tensor.matmul(
                                    o_ps[f],
                                    w2_sb[:, j, fo * 256:(fo + 1) * 256]
                                    .rearrange("p (two f) -> p two f", two=2),
                                    g_sb[:, 2 * j:2 * j + 2, :],
                                    start=(j == 0), stop=(j == FC // 2 - 1),
                                    perf_mode=DR)
                        for f in range(4):
                            fo = half * 4 + f
                            t = sm2.tile([128, 512], f32, tag="obias")
                            # w2 host-scaled by 16: o = o_ps/16 + bb2
                            nc.scalar.activation(t, o_ps[f], AF.Identity,
                                                 scale=0.0625,
                                                 bias=bb2_sb[:, fo:fo + 1])
                            # h = g1*cen + b1 (deferred LN1 affine)
                            h_aff = sm2.tile([128, 512], f32, tag="h_aff")
                            nc.scalar.activation(h_aff, hT[:, fo, :], AF.Identity,
                                                 scale=g1_sb[:, fo:fo + 1],
                                                 bias=b1_sb[:, fo:fo + 1])
                            nc.vector.tensor_tensor(r2T[:, fo, :], t, h_aff, OP.add)
                            # LN2 prep folded in: bf16 copy + square + partial sums
                            rb2 = r2B[:, fo, :]
                            nc.vector.tensor_copy(rb2, r2T[:, fo, :])
                            sq2 = sm2.tile([128, 512], bf16, tag="sq2")
                            nc.vector.tensor_tensor(sq2, rb2, rb2, OP.mult)
                            nc.tensor.matmul(s2_ps, ones_bf, rb2,
                                             start=(fo == 0), stop=(fo == D // 128 - 1))
                            nc.tensor.matmul(q2_ps, ones_bf, sq2,
                                             start=(fo == 0), stop=(fo == D // 128 - 1))

                  # ---- LN2 stats + normalize -> out (chunked DMA) ----
                  with tc.tile_pool(name="ln2out", bufs=3) as ln2out, \
                       tc.tile_pool(name="psM2", bufs=1, space="PSUM") as psM2:
                      mu = sm.tile([1, LQ], f32, tag="ln_mu")
                      nc.scalar.activation(mu, s2_ps, AF.Copy, scale=1.0 / D)
                      msq = sm.tile([1, LQ], f32, tag="ln_msq")
                      nc.scalar.activation(msq, q2_ps, AF.Copy, scale=1.0 / D)
                      var = sm.tile([1, LQ], f32, tag="ln_var")
                      nc.vector.tensor_tensor(var, mu, mu, OP.mult)
                      nc.vector.tensor_tensor(var, msq, var, OP.subtract)
                      std = sm.tile([1, LQ], f32, tag="ln_std")
                      nc.scalar.activation(std, var, AF.Sqrt, bias=eps_t)
                      mrrow = sm.tile([1, 2 * LQ], f32, tag="ln_mrrow")
                      nc.vector.reciprocal_approx_fast(mrrow[:, LQ:2 * LQ], std)
                      nc.vector.tensor_copy(mrrow[:, 0:LQ], mu)
                      mr_ps = psM2.tile([128, 2 * LQ], f32, tag="ln_mrps")
                      for j in range(2):
                          nc.tensor.matmul(mr_ps[:, j * LQ:(j + 1) * LQ], ones_row,
                                           mrrow[:, j * LQ:(j + 1) * LQ],
                                           start=True, stop=True)
                      mr16b = ln2out.tile([128, 2 * LQ], bf16, tag="mr16b")
                      nc.vector.tensor_copy(mr16b, mr_ps)
                      for c in range(DC):
                          cen = ln2out.tile([128, LQ], bf16, tag="ln_cen")
                          nc.vector.tensor_tensor(cen, r2B[:, c, :],
                                                  mr16b[:, 0:LQ], OP.subtract)
                          nc.vector.tensor_tensor(cen, cen,
                                                  mr16b[:, LQ:2 * LQ], OP.mult)
                          oc = ln2out.tile([128, LQ], f32, tag="ln_oc")
                          nc.scalar.activation(oc, cen, AF.Identity,
                                               scale=g2_sb[:, c:c + 1], bias=b2_sb[:, c:c + 1])
                          nc.sync.dma_start(out_v[:, c, :], oc)

    nc.compile()
    return nc


def _get_nc():
    if "nc" not in _cache:
        _cache["nc"] = _build_nc()
    return _cache["nc"]


def _host_prep(inputs):
    x = np.asarray(inputs["x"], np.float32)
    wq = np.asarray(inputs["wq"], np.float32)
    wk = np.asarray(inputs["wk"], np.float32)
    wo = np.asarray(inputs["wo"], np.float32)
    g1 = np.asarray(inputs["g1"], np.float32)
    b1 = np.asarray(inputs["b1"], np.float32)
    w1 = np.asarray(inputs["w1"], np.float32)
    bb1 = np.asarray(inputs["bb1"], np.float32)
    w2 = np.asarray(inputs["w2"], np.float32)
    bb2 = np.asarray(inputs["bb2"], np.float32)
    g2 = np.asarray(inputs["g2"], np.float32)
    b2 = np.asarray(inputs["b2"], np.float32)

    idx = np.arange(D)
    perm = (idx % HD) * NH + (idx // HD)  # f' = h*64+d  ->  old f = d*16+h

    def bf(a):
        return np.ascontiguousarray(a).astype(BF16NP)

    def f8(a):
        return np.ascontiguousarray(a).astype(F8NP)

    # w1 x8 / w2 x16 / wq,wk x8: power-of-2 pre-scales keep fp8 weights out of
    # the subnormal range; compensated on-device (gelu/obias/exp scale args,
    # kaug ones column = 8).  LN1's gamma/beta fold into w1/bb1 (z = cen@w1g
    # + bb1') so the device feeds raw cen into the FFN.
    w1g = w1 * g1[:, None]
    bb1 = bb1 + b1 @ w1
    w1t = (8.0 * w1g).reshape(DC, 128, FC, 128).transpose(2, 1, 0, 3).reshape(FC, 128, D)
    # w2p[j, p, fc*256 + i*128 + c] = 16*w2[(2j+i)*128+p, fc*128+c]
    w2p = (16.0 * w2).reshape(FC // 2, 2, 128, DC, 128).transpose(0, 2, 3, 1, 4) \
        .reshape(FC // 2, 128, 2 * D)
    def coblock(a):  # [K, F] -> [F-chunk, K-part, K-chunk-major cols]
        return a.reshape(DC, 128, DC, 128).transpose(2, 1, 0, 3).reshape(DC, 128, D)

    shared = {
        "wq": f8(coblock(8.0 * wq[:, perm])),
        "wk": f8(coblock(8.0 * wk[:, perm])),
        "wo": f8(coblock(wo[perm, :])),
        "w1": f8(w1t), "w2": f8(w2p),
        "ident": bf(np.eye(128, dtype=np.float32)),
        "bb1": bb1, "bb2": bb2, "g1": g1, "b1": b1, "g2": g2, "b2": b2,
    }
    in_maps = []
    for c in range(NCORES):
        b, q0 = c // (NCORES // B), (c % (NCORES // B)) * LQ
        xT = np.ascontiguousarray(x[b].T)
        m = dict(shared)
        m["xb"] = f8(np.ascontiguousarray(
            xT.reshape(D, 4, 512).transpose(1, 0, 2)))
        m["xqb"] = f8(xT[:, q0:q0 + LQ])
        m["xq"] = np.ascontiguousarray(xT[:, q0:q0 + LQ])
        in_maps.append(m)
    return in_maps


def kernel(**inputs):
    global LAST_RESULTS
    from concourse.bass_utils import run_bass_kernel_spmd

    nc = _get_nc()
    in_maps = _host_prep(inputs)
    res = run_bass_kernel_spmd(nc, in_maps, core_ids=list(range(NCORES)))
    LAST_RESULTS = res
    out = np.empty((B, L, D), np.float32)
    for c in range(NCORES):
        b, q0 = c // (NCORES // B), (c % (NCORES // B)) * LQ
        out[b, q0:q0 + LQ, :] = res.results[c]["out"].T
    return out


# revision 53
# speedup vs baseline: 1.1164x; 1.0268x over previous
"""Trainium2 Bass kernel for nn_EncoderLayer (B=2, L=2048, D=1024, 16 heads, FFN 4096).

Strategy: sequence-parallel over the 8 cores (core c owns batch c//4, query rows
(c%4)*512 .. +512).  Each core recomputes the full K projection for its batch,
which avoids all collectives; everything else is local.

v4 (485us -> ~310-325us HW): fp8 (e4m3) DoubleRow matmuls for q/k projections,
attn@K, wo, w1 AND w2 — 2x PE throughput (256-wide contraction per 512-cycle
instruction; verified on hw: same 216ns issue rate as a 128-wide bf16 matmul).
Scores stay bf16 (output-column-bound, fp8 gives no gain).  DoubleRow
stationaries must be contiguous [p,256] (walrus ISA check), hence the
co-blocked wq/wk/wo host layouts, the [128, MC/2, NH, 256] kaug layout with
m-chunk pairs adjacent per head, and the pair-interleaved w2 layout.

Precision plan (measured rel err ~1.0e-2 vs 2e-2 gate, matches numpy sim):
weights pre-scaled by powers of 2 (wq,wk,w1 x8; w2 x16) to avoid fp8
subnormals, compensated for free in ACT scale args (exp 1/512, gelu 1/8,
obias 1/16) and the kaug ones-column (=8, cancelling wk's x8 in the softmax
denominator).  Residual/LN paths stay fp32; attention fp8 error is diluted
~100x because the (faithfully reproduced) attn@K-instead-of-V bug makes
attn_out ~1% of x.

Schedule: per-head-pair pipeline (K-proj chunk co -> PE transposes -> scores/
exp/attn@K for heads 2co,2co+1) so ACT exp (~143us total, the attention-phase
floor) fills from ~6us in; PE projection work hides in ACT-bound slack.
Softmax denominators: per-pair reciprocal + DRAM partition-broadcast hidden
under the next pair's compute; the last pair uses a PE selector-matmul
broadcast (psK is free) to keep the DMA round-trip off the critical path into
wo.  LN1's gamma/beta fold into w1/bb1 on the host, so the LN1->FFN seam only
needs sub/mult/fp8-cast per chunk; the residual affine runs inside the FFN
window where ACT is idle.  Both LN normalizes run at bf16 DVE width from the
bf16 copies already made for the stats sums (r1B/r2B) — this closed the LN1
seam to ~2us and costs ~0.1% extra rel err.  x-residual and first w1 tiles
prefetch during attention/wo.  The kaug padding memset is split per head-pair
inside the loop — one upfront [128,8,32,63] memset stalled the in-order DVE
queue 13.5us ahead of the critical kT/qT casts (worth ~35us end-to-end; a
65-col unpadded DoubleRow stationary fails the walrus ISA check, so padding
stays).  Remaining known costs: ~14us LN2 tail (ACT affine + serial stats
chain + output DMA drain), ~7us runtime-prologue startup, DVFS throttling
(util limit ~0.72-0.78 under fp8 load; run-to-run noise +-10us).
"""

import sys
sys.setrecursionlimit(200000)
import numpy as np
import ml_dtypes

B, L, D, NH, HD, FF = 2, 2048, 1024, 16, 64, 4096
LQ = 512  # query rows per core
NCORES = 8
EPS = 1e-5
DC = D // 128  # 8 feature chunks
MC = L // 128  # 16 key chunks
FC = FF // 128  # 32 ffn chunks
BF16NP = ml_dtypes.bfloat16
F8NP = ml_dtypes.float8_e4m3

_cache = {}
LAST_RESULTS = None


def _build_nc():
    import concourse.bass as bass
    import concourse.tile as tile
    from concourse import bacc, mybir
    from contextlib import ExitStack

    f32 = mybir.dt.float32
    bf16 = mybir.dt.bfloat16
    f8 = mybir.dt.float8e4
    AF = mybir.ActivationFunctionType
    OP = mybir.AluOpType
    DR = mybir.MatmulPerfMode.DoubleRow

    nc = bacc.Bacc("TRN2", debug=False, target_bir_lowering=False)

    # ---- DRAM I/O ----
    xb_d = nc.dram_tensor("xb", [4, D, 512], f8, kind="ExternalInput").ap()
    xqb_d = nc.dram_tensor("xqb", [D, LQ], f8, kind="ExternalInput").ap()
    xq_d = nc.dram_tensor("xq", [D, LQ], f32, kind="ExternalInput").ap()
    wq_d = nc.dram_tensor("wq", [DC, 128, D], f8, kind="ExternalInput").ap()
    wk_d = nc.dram_tensor("wk", [DC, 128, D], f8, kind="ExternalInput").ap()
    wo_d = nc.dram_tensor("wo", [DC, 128, D], f8, kind="ExternalInput").ap()
    w1_d = nc.dram_tensor("w1", [FC, 128, D], f8, kind="ExternalInput").ap()
    w2_d = nc.dram_tensor("w2", [FC // 2, 128, 2 * D], f8, kind="ExternalInput").ap()
    ident_d = nc.dram_tensor("ident", [128, 128], bf16, kind="ExternalInput").ap()
    bb1_d = nc.dram_tensor("bb1", [FF], f32, kind="ExternalInput").ap()
    bb2_d = nc.dram_tensor("bb2", [D], f32, kind="ExternalInput").ap()
    g1_d = nc.dram_tensor("g1", [D], f32, kind="ExternalInput").ap()
    b1_d = nc.dram_tensor("b1", [D], f32, kind="ExternalInput").ap()
    g2_d = nc.dram_tensor("g2", [D], f32, kind="ExternalInput").ap()
    b2_d = nc.dram_tensor("b2", [D], f32, kind="ExternalInput").ap()
    out_d = nc.dram_tensor("out", [D, LQ], f32, kind="ExternalOutput").ap()

    xqb_v = xqb_d.rearrange("(c p) l -> p c l", p=128)
    xq_v = xq_d.rearrange("(c p) l -> p c l", p=128)
    bb1_v = bb1_d.rearrange("(c p) -> p c", p=128)
    bb2_v = bb2_d.rearrange("(c p) -> p c", p=128)
    g1_v = g1_d.rearrange("(c p) -> p c", p=128)
    b1_v = b1_d.rearrange("(c p) -> p c", p=128)
    g2_v = g2_d.rearrange("(c p) -> p c", p=128)
    b2_v = b2_d.rearrange("(c p) -> p c", p=128)
    out_v = out_d.rearrange("(c p) l -> p c l", p=128)

    with tile.TileContext(nc, pool_alloc_mode="queue") as tc, ExitStack() as top:
        consts = top.enter_context(tc.tile_pool(name="consts", bufs=1))
        dramsc = top.enter_context(tc.tile_pool(name="dramsc", bufs=2, space="DRAM"))

        sm = top.enter_context(tc.tile_pool(name="smalls", bufs=1))
        sm2 = top.enter_context(tc.tile_pool(name="smalls2", bufs=2))

        with tc.tile_pool(name="mid", bufs=1) as mid:
            hT = mid.tile([128, DC, LQ], bf16, tag="hT")
            hb = mid.tile([128, DC, LQ], f8, tag="hb")

            with tc.tile_pool(name="kq", bufs=1) as kq:
                kT = kq.tile([128, DC, L], bf16, tag="kT")
                # kaug[p, mj, h, i*128 + j]: m-chunk pair mj, head h, k-tile i
                # (m = 2*mj+i), col j in [0:64] = head dims, 64 = ones, rest pad
                kaug = kq.tile([128, MC // 2, NH, 256], f8, tag="kaug")
                qT = kq.tile([128, DC, LQ], bf16, tag="qT")
                ctxT = kq.tile([128, DC, LQ], f8, tag="ctxT")

                # ---- Phase 1+2: interleaved projections + attention ----
                with tc.tile_pool(name="p1", bufs=1) as p1, \
                     tc.tile_pool(name="p1w", bufs=1) as p1w, \
                     tc.tile_pool(name="epool", bufs=2) as epool, \
                     tc.tile_pool(name="cpool", bufs=2) as cpool, \
                     tc.tile_pool(name="wop", bufs=1) as wop, \
                     tc.tile_pool(name="psK", bufs=2, space="PSUM") as psK, \
                     tc.tile_pool(name="psT", bufs=1, space="PSUM") as psT, \
                     tc.tile_pool(name="psS", bufs=2, space="PSUM") as psS, \
                     tc.tile_pool(name="psU", bufs=1, space="PSUM") as psU:
    # chunk-0 k-path inputs first so the tensor engine starts early;
                    # the rest of the weight chunks stream behind xb so
                    # head-pair co's inputs land just in time
                    wq_sb = p1w.tile([128, DC, D], f8, tag="wproj")
                    wk_sb = p1w.tile([128, DC, D], f8, tag="wproj_k")
                    xb = p1.tile([128, 4, DC, 512], f8, tag="xb")
                    xqb = p1.tile([128, DC, LQ], f8, tag="xqb")
                    nc.sync.dma_start(xqb, xqb_v)
                    nc.sync.dma_start(wq_sb[:, 0, :], wq_d[0])
                    nc.sync.dma_start(wk_sb[:, 0, :], wk_d[0])
                    for mt in range(4):
                        nc.sync.dma_start(
                            xb[:, mt, :, :],
                            xb_d[mt].rearrange("(c p) m -> p c m", p=128))
                    ident = consts.tile([128, 128], bf16, tag="ident")
                    nc.sync.dma_start(ident, ident_d)
                    for co in range(1, DC):
                        nc.sync.dma_start(wq_sb[:, co, :], wq_d[co])
                        nc.sync.dma_start(wk_sb[:, co, :], wk_d[co])

    # constants (small DMAs, off the critical path)
                    ones_bf = consts.tile([128, 1], bf16, tag="ones")
                    nc.vector.memset(ones_bf, 1.0)
                    # selector rows: sel<s> broadcasts a [1,512] row onto
                    # partitions s*64..s*64+64 via a PE matmul
                    sel0 = consts.tile([1, 128], bf16, tag="sel0")
                    nc.vector.memset(sel0, 0.0)
                    nc.vector.memset(sel0[:, 0:64], 1.0)
                    sel1 = consts.tile([1, 128], bf16, tag="sel1")
                    nc.vector.memset(sel1, 0.0)
                    nc.vector.memset(sel1[:, 64:128], 1.0)
                    ones_row = consts.tile([1, 128], f32, tag="ones_row")
                    nc.vector.memset(ones_row, 1.0)
                    eps_t = consts.tile([1, 1], f32, tag="eps")
                    nc.vector.memset(eps_t, EPS)
                    bb1_sb = consts.tile([128, FC], f32, tag="bb1")
                    nc.sync.dma_start(bb1_sb, bb1_v)
                    bb2_sb = consts.tile([128, DC], f32, tag="bb2")
                    nc.sync.dma_start(bb2_sb, bb2_v)
                    g1_sb = consts.tile([128, DC], f32, tag="g1")
                    nc.sync.dma_start(g1_sb, g1_v)
                    b1_sb = consts.tile([128, DC], f32, tag="b1")
                    nc.sync.dma_start(b1_sb, b1_v)
                    g2_sb = consts.tile([128, DC], f32, tag="g2")
                    nc.sync.dma_start(g2_sb, g2_v)
                    b2_sb = consts.tile([128, DC], f32, tag="b2")
                    nc.sync.dma_start(b2_sb, b2_v)
                    kaug_b = kaug.rearrange("p mj h (two f) -> p mj (h two) f",
                                            two=2)
                    # ones column = 8.0: wk is host-scaled by 8, so kT holds
                    # 8*k; den row becomes 8*sum(e), cancelling the 8 in ctx
                    nc.vector.memset(kaug_b[:, :, :, 64:65], 8.0)

                    wo_sb = wop.tile([128, DC, D], f8, tag="wo_sb")
                    scd = dramsc.tile([NH, LQ], bf16, tag="rec_sc")

                    for co in range(DC):
                        if co == 2:
                            # prefetch wo once the input stream has drained
                            for cw in range(DC):
                                nc.sync.dma_start(wo_sb[:, cw, :], wo_d[cw])
                        # ---- q chunk co ----
                        psq = psK.tile([128, 512], f32, tag="psk")
                        for cp in range(DC // 2):
                            nc.tensor.matmul(
                                psq,
                                wq_sb[:, co, cp * 256:(cp + 1) * 256]
                                .rearrange("p (two f) -> p two f", two=2),
                                xqb[:, 2 * cp:2 * cp + 2, :],
                                start=(cp == 0), stop=(cp == DC // 2 - 1),
                                perf_mode=DR)
                        nc.vector.tensor_copy(qT[:, co, :], psq)

                        # ---- k chunk co over full L ----
                        for mt in range(4):
                            ps = psK.tile([128, 512], f32, tag="psk")
                            for cp in range(DC // 2):
                                nc.tensor.matmul(
                                    ps,
                                    wk_sb[:, co, cp * 256:(cp + 1) * 256]
                                    .rearrange("p (two f) -> p two f", two=2),
                                    xb[:, mt, 2 * cp:2 * cp + 2, :],
                                    start=(cp == 0), stop=(cp == DC // 2 - 1),
                                    perf_mode=DR)
                            nc.vector.tensor_copy(
                                kT[:, co, mt * 512:(mt + 1) * 512], ps)

                        # ---- transposes -> kaug for heads 2co, 2co+1 ----
                        for g in range(2):
                            pt = psT.tile([128, 1024], bf16, tag="pt")
                            for j in range(8):
                                mi = g * 8 + j
                                nc.tensor.transpose(
                                    pt[:, j * 128:(j + 1) * 128],
                                    kT[:, co, mi * 128:(mi + 1) * 128], ident)
                            ptv = pt.rearrange("p (m he) -> p m he", he=128)
                            for s in range(2):
                                for i in range(2):
                                    # m-chunks g*8+i, g*8+i+2, ... (parity i)
                                    nc.vector.tensor_copy(
                                        kaug[:, g * 4:(g + 1) * 4, 2 * co + s,
                                             i * 128:i * 128 + 64],
                                        ptv[:, i::2, s * 64:(s + 1) * 64])
                        # zero this pair's kaug padding off the critical DVE
                        # path (one big upfront memset stalled the queue 13us)
                        nc.vector.memset(
                            kaug_b[:, :, 4 * co:4 * co + 4, 65:128], 0.0)

                        # ---- heads 2co, 2co+1 ----
                        cT = cpool.tile([128, LQ], bf16, tag="cT")
                        den_bc = cpool.tile([128, LQ], bf16, tag="den_bc")
                        rec_pair = []
                        for s in range(2):
                            h = 2 * co + s
                            poff = 64 * s
                            e = epool.tile([128, MC, LQ], f8, tag="E")
                            for mt in range(MC // 2):
                                st = psS.tile([128, 1024], f32, tag="st")
                                for j in range(2):
                                    mi = mt * 2 + j
                                    nc.tensor.matmul(
                                        st[:, j * 512:(j + 1) * 512],
                                        kT[poff:poff + 64, co,
                                           mi * 128:(mi + 1) * 128],
                                        qT[poff:poff + 64, co, :],
                                        start=True, stop=True)
                                # wq,wk host-scaled by 8 => scores are 64x;
                                # fold 1/sqrt(HD)/64 = 1/512 into the exp
                                nc.scalar.activation(
                                    e[:, mt * 2:(mt + 1) * 2, :]
                                    .rearrange("p a b -> p (a b)"),
                                    st, AF.Exp, scale=1.0 / 512.0)
                            u = psU.tile([128, 512], f32, tag="u")
                            for mj in range(MC // 2):
                                nc.tensor.matmul(
                                    u, kaug[:, mj, h, :]
                                    .rearrange("p (two f) -> p two f", two=2),
                                    e[:, 2 * mj:2 * mj + 2, :],
                                    start=(mj == 0), stop=(mj == MC // 2 - 1),
                                    perf_mode=DR)
                            nc.vector.tensor_copy(cT[poff:poff + 64, :],
                                                  u[0:64, :])
                            drow = sm2.tile([1, LQ], f32, tag="drow")
                            nc.vector.tensor_copy(drow, u[64:65, :])
                            rec32 = sm2.tile([1, LQ], f32, tag="rec32")
                            nc.vector.reciprocal_approx_fast(rec32, drow)
                            rec16 = sm2.tile([1, LQ], bf16, tag="rec16")
                            nc.vector.tensor_copy(rec16, rec32)
                            if co < DC - 1:
                                nc.sync.dma_start(scd[h:h + 1, :], rec16)
                                nc.sync.dma_start(
                                    den_bc[poff:poff + 64, :],
                                    scd[h:h + 1, :].partition_broadcast(64))
                            else:
                                # last pair: PE selector broadcast (psK is
                                # free) -- keeps the DMA round-trip off the
                                # critical path into the wo loop
                                rec_pair.append(rec16)
                        if co < DC - 1:
                            nc.vector.tensor_tensor(ctxT[:, co, :], cT, den_bc,
                                                    OP.mult)
                        else:
                            den_ps = psK.tile([128, LQ], f32, tag="psk")
                            nc.tensor.matmul(den_ps, sel0, rec_pair[0],
                                             start=True, stop=False)
                            nc.tensor.matmul(den_ps, sel1, rec_pair[1],
                                             start=False, stop=True)
                            nc.vector.tensor_tensor(ctxT[:, co, :], cT, den_ps,
                                                    OP.mult)

                # ---- attn_out + residual -> r1T, with LN1 prep folded in ----
                with tc.tile_pool(name="r1p", bufs=1) as r1p, \
                     tc.tile_pool(name="psL1", bufs=1, space="PSUM") as psL1, \
                     tc.tile_pool(name="cen1p", bufs=2) as cen1p, \
                     tc.tile_pool(name="psM1", bufs=1, space="PSUM") as psM1:
                    s1_ps = psL1.tile([1, LQ], f32, tag="ln1_sum_r")
                    q1_ps = psL1.tile([1, LQ], f32, tag="ln1_sum_s")
                    r1T = r1p.tile([128, DC, LQ], f32, tag="r1T")
                    r1B = r1p.tile([128, DC, LQ], bf16, tag="r1B")
                    xq_all = r1p.tile([128, DC, LQ], f32, tag="xq_all")
                    for cw in range(DC):
                        nc.sync.dma_start(xq_all[:, cw, :], xq_v[:, cw, :])
                    with tc.tile_pool(name="psB", bufs=4, space="PSUM") as psB:
                        for f in range(DC):
                            ps = psB.tile([128, 512], f32, tag="ao")
                            for cp in range(DC // 2):
                                nc.tensor.matmul(
                                    ps,
                                    wo_sb[:, f, cp * 256:(cp + 1) * 256]
                                    .rearrange("p (two f) -> p two f", two=2),
                                    ctxT[:, 2 * cp:2 * cp + 2, :],
                                    start=(cp == 0), stop=(cp == DC // 2 - 1),
                                    perf_mode=DR)
                            nc.vector.tensor_tensor(r1T[:, f, :], ps,
                                                    xq_all[:, f, :], OP.add)
                            rb1 = r1B[:, f, :]
                            nc.vector.tensor_copy(rb1, r1T[:, f, :])
                            sq1 = sm2.tile([128, 512], bf16, tag="sq1")
                            nc.vector.tensor_tensor(sq1, rb1, rb1, OP.mult)
                            nc.tensor.matmul(s1_ps, ones_bf, rb1,
                                             start=(f == 0), stop=(f == DC - 1))
                            nc.tensor.matmul(q1_ps, ones_bf, sq1,
                                             start=(f == 0), stop=(f == DC - 1))

                    # prefetch the first w1 tiles so the FFN stream starts
                    # as soon as hb chunks appear
                    w1_pre = []
                    for i in range(2):
                        w1t = sm2.tile([128, D], f8, tag="w1pre")
                        nc.sync.dma_start(w1t, w1_d[i])
                        w1_pre.append(w1t)

                    # ---- LN1 stats + normalize (chunkwise) -> hT, hb ----
                    mu = sm.tile([1, LQ], f32, tag="ln_mu")
                    nc.scalar.activation(mu, s1_ps, AF.Copy, scale=1.0 / D)
                    msq = sm.tile([1, LQ], f32, tag="ln_msq")
                    nc.scalar.activation(msq, q1_ps, AF.Copy, scale=1.0 / D)
                    var = sm.tile([1, LQ], f32, tag="ln_var")
                    nc.vector.tensor_tensor(var, mu, mu, OP.mult)
                    nc.vector.tensor_tensor(var, msq, var, OP.subtract)
                    std = sm.tile([1, LQ], f32, tag="ln_std")
                    nc.scalar.activation(std, var, AF.Sqrt, bias=eps_t)
                    mrrow = sm.tile([1, 2 * LQ], f32, tag="ln_mrrow")
                    nc.vector.reciprocal_approx_fast(mrrow[:, LQ:2 * LQ], std)
                    nc.vector.tensor_copy(mrrow[:, 0:LQ], mu)
                    mr_ps = psM1.tile([128, 2 * LQ], f32, tag="ln_mrps")
                    for j in range(2):
                        nc.tensor.matmul(mr_ps[:, j * LQ:(j + 1) * LQ], ones_row,
                                         mrrow[:, j * LQ:(j + 1) * LQ],
                                         start=True, stop=True)
                    mu_bc, rstd_bc = mr_ps[:, 0:LQ], mr_ps[:, LQ:2 * LQ]
                    # hT holds raw cen = (r1-mu)*rstd (bf16); g1/b1 fold into
                    # w1 on the host (w1g, bb1') for the z path, and apply
                    # via an ACT affine inside the FFN window for the residual.
                    # Normalize runs at bf16 DVE width from the r1B copy.
                    mr16 = cen1p.tile([128, 2 * LQ], bf16, tag="mr16")
                    nc.vector.tensor_copy(mr16, mr_ps)
                    for c in range(DC):
                        cen = cen1p.tile([128, LQ], bf16, tag="ln_cen")
                        nc.vector.tensor_tensor(cen, r1B[:, c, :],
                                                mr16[:, 0:LQ], OP.subtract)
                        nc.vector.tensor_tensor(hT[:, c, :], cen,
                                                mr16[:, LQ:2 * LQ], OP.mult)
                        nc.scalar.activation(hb[:, c, :], hT[:, c, :], AF.Copy)
            # ---- Phase 3: FFN ----
            with tc.tile_pool(name="ffn", bufs=1) as ffn, \
                 tc.tile_pool(name="w1stream", bufs=4) as w1stream, \
                 tc.tile_pool(name="w2pool", bufs=1) as w2pool:
                g_sb = ffn.tile([128, FC, LQ], f8, tag="g")
                r2T = ffn.tile([128, DC, LQ], f32, tag="r2T")
                r2B = ffn.tile([128, DC, LQ], bf16, tag="r2B")
                w2_sb = w2pool.tile([128, FC // 2, 2 * D], f8, tag="w2_sb")

                with tc.tile_pool(name="psL2", bufs=1, space="PSUM") as psL2:
                  s2_ps = psL2.tile([1, LQ], f32, tag="ln2_sum_r")
                  q2_ps = psL2.tile([1, LQ], f32, tag="ln2_sum_s")
                  with tc.tile_pool(name="psZO", bufs=1, space="PSUM") as psZO:
                    for half in range(2):
                        o_ps = [psZO.tile([128, 512], f32, tag=f"o{f}", name=f"o_ps{f}")
                                for f in range(4)]
                        for j in range(FC // 2):
                            if half == 0:
                                for i in (2 * j, 2 * j + 1):
                                    if i < 2:
                                        w1t = w1_pre[i]
                                    else:
                                        w1t = w1stream.tile([128, D], f8, tag="w1t")
                                        nc.sync.dma_start(w1t, w1_d[i])
                                    zt = psZO.tile([128, 512], f32, tag=f"zt{i % 2}",
                                                   name=f"zt{i % 2}")
                                    for cp in range(DC // 2):
                                        nc.tensor.matmul(
                                            zt,
                                            w1t[:, cp * 256:(cp + 1) * 256]
                                            .rearrange("p (two f) -> p two f", two=2),
                                            hb[:, 2 * cp:2 * cp + 2, :],
                                            start=(cp == 0), stop=(cp == DC // 2 - 1),
                                            perf_mode=DR)
                                    # w1 host-scaled by 8: z = zt/8 + bb1
                                    nc.scalar.activation(g_sb[:, i, :], zt, AF.Gelu,
                                                         scale=0.125,
                                                         bias=bb1_sb[:, i:i + 1])
                                nc.sync.dma_start(w2_sb[:, j, :], w2_d[j])
                            for f in range(4):
                                fo = half * 4 + f
                                nc.tensor.matmul(
                                    o_ps[f],
                                    w2_sb[:, j, fo * 256:(fo + 1) * 256]
                                    .rearrange("p (two f) -> p two f", two=2),
                                    g_sb[:, 2 * j:2 * j + 2, :],
                                    start=(j == 0), stop=(j == FC // 2 - 1),
                                    perf_mode=DR)
                        for f in range(4):
                            fo = half * 4 + f
                            t = sm2.tile([128, 512], f32, tag="obias")
                            # w2 host-scaled by 16: o = o_ps/16 + bb2
                            nc.scalar.activation(t, o_ps[f], AF.Identity,
                                                 scale=0.0625,
                                                 bias=bb2_sb[:, fo:fo + 1])
                            # h = g1*cen + b1 (deferred LN1 affine)
                            h_aff = sm2.tile([128, 512], f32, tag="h_aff")
                            nc.scalar.activation(h_aff, hT[:, fo, :], AF.Identity,
                                                 scale=g1_sb[:, fo:fo + 1],
                                                 bias=b1_sb[:, fo:fo + 1])
                            nc.vector.tensor_tensor(r2T[:, fo, :], t, h_aff, OP.add)
                            # LN2 prep folded in: bf16 copy + square + partial sums
                            rb2 = r2B[:, fo, :]
                            nc.vector.tensor_copy(rb2, r2T[:, fo, :])
                            sq2 = sm2.tile([128, 512], bf16, tag="sq2")
                            nc.vector.tensor_tensor(sq2, rb2, rb2, OP.mult)
                            nc.tensor.matmul(s2_ps, ones_bf, rb2,
                                             start=(fo == 0), stop=(fo == D // 128 - 1))
                            nc.tensor.matmul(q2_ps, ones_bf, sq2,
                                             start=(fo == 0), stop=(fo == D // 128 - 1))

                  # ---- LN2 stats + normalize -> out (chunked DMA) ----
                  with tc.tile_pool(name="ln2out", bufs=3) as ln2out, \
                       tc.tile_pool(name="psM2", bufs=1, space="PSUM") as psM2:
                      mu = sm.tile([1, LQ], f32, tag="ln_mu")
                      nc.scalar.activation(mu, s2_ps, AF.Copy, scale=1.0 / D)
                      msq = sm.tile([1, LQ], f32, tag="ln_msq")
                      nc.scalar.activation(msq, q2_ps, AF.Copy, scale=1.0 / D)
                      var = sm.tile([1, LQ], f32, tag="ln_var")
                      nc.vector.tensor_tensor(var, mu, mu, OP.mult)
                      nc.vector.tensor_tensor(var, msq, var, OP.subtract)
                      std = sm.tile([1, LQ], f32, tag="ln_std")
                      nc.scalar.activation(std, var, AF.Sqrt, bias=eps_t)
                      mrrow = sm.tile([1, 2 * LQ], f32, tag="ln_mrrow")
                      nc.vector.reciprocal_approx_fast(mrrow[:, LQ:2 * LQ], std)
                      nc.vector.tensor_copy(mrrow[:, 0:LQ], mu)
                      mr_ps = psM2.tile([128, 2 * LQ], f32, tag="ln_mrps")
                      for j in range(2):
                          nc.tensor.matmul(mr_ps[:, j * LQ:(j + 1) * LQ], ones_row,
                                           mrrow[:, j * LQ:(j + 1) * LQ],
                                           start=True, stop=True)
                      mr16b = ln2out.tile([128, 2 * LQ], bf16, tag="mr16b")
                      nc.vector.tensor_copy(mr16b, mr_ps)
                      for c in range(DC):
                          cen = ln2out.tile([128, LQ], bf16, tag="ln_cen")
                          nc.vector.tensor_tensor(cen, r2B[:, c, :],
                                                  mr16b[:, 0:LQ], OP.subtract)
                          nc.vector.tensor_tensor(cen, cen,
                                                  mr16b[:, LQ:2 * LQ], OP.mult)
                          oc = ln2out.tile([128, LQ], f32, tag="ln_oc")
                          nc.scalar.activation(oc, cen, AF.Identity,
                                               scale=g2_sb[:, c:c + 1], bias=b2_sb[:, c:c + 1])
                          nc.sync.dma_start(out_v[:, c, :], oc)

    nc.compile()
    return nc


def _get_nc():
    if "nc" not in _cache:
        _cache["nc"] = _build_nc()
    return _cache["nc"]


def _host_prep(inputs):
    x = np.asarray(inputs["x"], np.float32)
    wq = np.asarray(inputs["wq"], np.float32)
    wk = np.asarray(inputs["wk"], np.float32)
    wo = np.asarray(inputs["wo"], np.float32)
    g1 = np.asarray(inputs["g1"], np.float32)
    b1 = np.asarray(inputs["b1"], np.float32)
    w1 = np.asarray(inputs["w1"], np.float32)
    bb1 = np.asarray(inputs["bb1"], np.float32)
    w2 = np.asarray(inputs["w2"], np.float32)
    bb2 = np.asarray(inputs["bb2"], np.float32)
    g2 = np.asarray(inputs["g2"], np.float32)
    b2 = np.asarray(inputs["b2"], np.float32)

    idx = np.arange(D)
    perm = (idx % HD) * NH + (idx // HD)  # f' = h*64+d  ->  old f = d*16+h

    def bf(a):
        return np.ascontiguousarray(a).astype(BF16NP)

    def f8(a):
        return np.ascontiguousarray(a).astype(F8NP)

    # w1 x8 / w2 x16 / wq,wk x8: power-of-2 pre-scales keep fp8 weights out of
    # the subnormal range; compensated on-device (gelu/obias/exp scale args,
    # kaug ones column = 8).  LN1's gamma/beta fold into w1/bb1 (z = cen@w1g
    # + bb1') so the device feeds raw cen into the FFN.
    w1g = w1 * g1[:, None]
    bb1 = bb1 + b1 @ w1
    w1t = (8.0 * w1g).reshape(DC, 128, FC, 128).transpose(2, 1, 0, 3).reshape(FC, 128, D)
    # w2p[j, p, fc*256 + i*128 + c] = 16*w2[(2j+i)*128+p, fc*128+c]
    w2p = (16.0 * w2).reshape(FC // 2, 2, 128, DC, 128).transpose(0, 2, 3, 1, 4) \
        .reshape(FC // 2, 128, 2 * D)
    def coblock(a):  # [K, F] -> [F-chunk, K-part, K-chunk-major cols]
        return a.reshape(DC, 128, DC, 128).transpose(2, 1, 0, 3).reshape(DC, 128, D)

    shared = {
        "wq": f8(coblock(8.0 * wq[:, perm])),
        "wk": f8(coblock(8.0 * wk[:, perm])),
        "wo": f8(coblock(wo[perm, :])),
        "w1": f8(w1t), "w2": f8(w2p),
        "ident": bf(np.eye(128, dtype=np.float32)),
        "bb1": bb1, "bb2": bb2, "g1": g1, "b1": b1, "g2": g2, "b2": b2,
    }
    in_maps = []
    for c in range(NCORES):
        b, q0 = c // (NCORES // B), (c % (NCORES // B)) * LQ
        xT = np.ascontiguousarray(x[b].T)
        m = dict(shared)
        m["xb"] = f8(np.ascontiguousarray(
            xT.reshape(D, 4, 512).transpose(1, 0, 2)))
        m["xqb"] = f8(xT[:, q0:q0 + LQ])
        m["xq"] = np.ascontiguousarray(xT[:, q0:q0 + LQ])
        in_maps.append(m)
    return in_maps


def kernel(**inputs):
    global LAST_RESULTS
    from concourse.bass_utils import run_bass_kernel_spmd

    nc = _get_nc()
    in_maps = _host_prep(inputs)
    res = run_bass_kernel_spmd(nc, in_maps, core_ids=list(range(NCORES)))
    LAST_RESULTS = res
    out = np.empty((B, L, D), np.float32)
    for c in range(NCORES):
        b, q0 = c // (NCORES // B), (c % (NCORES // B)) * LQ
        out[b, q0:q0 + LQ, :] = res.results[c]["out"].T
    return out
